# revision 25
# baseline (speedup 1.0000x reference)
"""Causal STFT kernel for Trainium2 (8 NeuronCores, data-parallel over batch).

Problem: x [16, 524288] f32 -> mag [16, 513, 2048] f32.
  Per batch: causal pad 1023 zeros on the left, frames of 1024 at hop 256
  (2048 frames), multiply by Hann-windowed DFT basis (1026 x 1024), take
  per-bin magnitude sqrt(clip(re^2 + im^2, 1e-12)).

Sharding: batch dim split 2 per core across 8 cores (SPMD, no collectives).

Default device strategy, MODE='fold' (~89.4 us HW time, ~3e-4 rel err;
the magnitude pipeline keeps sq/s tiles in fp16 so the DVE add runs in its
2x perf mode).  MODE='dif' is an experimental radix-2 decimation-in-
frequency variant (21 vs 36 matmuls per tile, host-side folds) that is
engine-balanced at ~45us but currently schedules to ~97us on HW.
  - Host relayouts each padded signal (pure layout, each element stored once
    per view): C_h[p, c] = xp[256c + 128h + p] and a partition-reversed copy
    D_g[p, c] = xp[256c - 128g - p], both fp16.  Frame t window position
    m = 128a + p is then C_{a&1}[p, t + (a>>1)], and position 1024 - m is
    D_{a&1}[p, t + 4 - (a>>1)].
  - The Hann-windowed DFT rows are symmetric (cos) / antisymmetric (sin)
    about the frame center, so DVE folds Fplus = C + D, Fminus = C - D
    halve the tensor-engine contraction to K = 512.  The m=0 fold slot has
    zero window weight and is repurposed for the self-paired center sample
    x[512] whose weight column is w2[:, 512]; this also packs the bin-512
    cos row in as a 513th M column of the cos weights.
  - TensorE (fp16): per (batch, 512-frame tile): 4 cos 128-bin tiles +
    1-row bin-512 tile + 4 sin tiles, each accumulating 4 K-chunks in PSUM.
  - ACT squares PSUM pairs, DVE adds them with the eps clip, ACT takes the
    sqrt, outputs stream out per 128x512 tile.  Bin 0 falls out of the
    all-zero sin_0 row; bin 512 is |re_512| via ACT Abs.
"""

import os
import sys

import numpy as np

for _p in ("/opt/trn_rl_repo",):
    if _p not in sys.path and os.path.isdir(_p):
        sys.path.insert(0, _p)

N_FFT = 1024
HOP = 256
CACHE = N_FFT - 1  # 1023 zeros of causal left pad
BATCH = 16
SAMPLES = HOP * 2048
L = 2048  # frames per batch
F = 513  # output bins per batch
NCORES = 8
BPC = BATCH // NCORES  # batches per core = 2
NCHUNK = (CACHE + SAMPLES + 1) // HOP  # 2052 chunks of 256 after padding
KT = N_FFT // 128  # 8 contraction tiles
NT = L // 512  # 4 frame tiles
QT = 4  # 4 (re, im) pair tiles of 128 bins

# matmul mode: 'dif' (radix-2 decimation-in-frequency, K=256, host folds),
# 'fold' (K=512 via window symmetry, fp16), or direct K=1024
# modes 'f32r' (full-rate fp32-in), 'f16', 'bf16', 'f32' (4x slow)
MODE = os.environ.get("STFT_MM_DTYPE", "fold")
# engine assignment knobs for the dif magnitude stage
POOL_SQ = int(os.environ.get("STFT_POOL_SQ", "2"))  # pair squares on gpsimd (0-4)
MERGE_OUT = os.environ.get("STFT_MERGE_OUT", "1") == "1"  # single out-DMA per (b,n)

_PROGRAM_CACHE = {}


def _mm_dtype(mybir):
    return {
        "f32r": mybir.dt.float32r,
        "f32": mybir.dt.float32,
        "f16": mybir.dt.float16,
        "bf16": mybir.dt.bfloat16,
    }[MODE]


def _np_w_dtype():
    import ml_dtypes

    return {
        "f32r": np.float32,
        "f32": np.float32,
        "f16": np.float16,
        "bf16": ml_dtypes.bfloat16,
    }[MODE]


def _build_program():
    import concourse.bacc as bacc
    import concourse.mybir as mybir
    import concourse.tile as tile

    DT = _mm_dtype(mybir)
    f32 = mybir.dt.float32
    needs_cast = MODE in ("f16", "bf16")

    nc = bacc.Bacc("TRN2", target_bir_lowering=False, debug=False)
    w_in = nc.declare_dram_parameter("w", [KT, 128, 1024], DT, isOutput=False)
    c_in = nc.declare_dram_parameter(
        "c", [BPC, 2, 128, NCHUNK], f32 if needs_cast else DT, isOutput=False
    )
    out = nc.declare_dram_parameter("out", [BPC, F, L], f32, isOutput=True)

    # column chunks for the signal loads: n-tile j only needs cols
    # [512j, 512j+516), so chunked DMA+cast lets matmuls start early.
    CB = [0, 516, 1032, 1548, NCHUNK]

    with tile.TileContext(nc) as tc:
        with (
            tc.tile_pool(name="wp", bufs=1) as wp,
            tc.tile_pool(name="cp", bufs=1) as cp,
            tc.tile_pool(name="castp", bufs=1) as castp,
            tc.tile_pool(name="ps", bufs=3, space="PSUM") as ps,
            tc.tile_pool(name="sqp", bufs=3) as sqp,
            tc.tile_pool(name="sp", bufs=3) as sp,
            tc.tile_pool(name="stp", bufs=3) as stp,
            tc.tile_pool(name="r512p", bufs=2) as r512p,
        ):
            w_sb = [None] * KT

            def load_w(k):
                wt = wp.tile([128, 1024], DT, name=f"w{k}")
                nc.sync.dma_start(wt[:], w_in[k])
                w_sb[k] = wt

            c_sb = [[None, None] for _ in range(BPC)]

            def load_c(b, chunks):
                for h in range(2):
                    if c_sb[b][h] is None:
                        c_sb[b][h] = cp.tile(
                            [128, NCHUNK], f32 if needs_cast else DT, name=f"c{b}{h}"
                        )
                        if needs_cast:
                            cast = castp.tile([128, NCHUNK], DT, name=f"cc{b}{h}")
                            c_sb[b][h] = (c_sb[b][h], cast)
                for j in chunks:
                    lo, hi = CB[j], CB[j + 1]
                    for h in range(2):
                        t = c_sb[b][h]
                        if needs_cast:
                            raw, cast = t
                            nc.sync.dma_start(raw[:, lo:hi], c_in[b, h, :, lo:hi])
                            nc.vector.tensor_copy(cast[:, lo:hi], raw[:, lo:hi])
                        else:
                            nc.sync.dma_start(t[:, lo:hi], c_in[b, h, :, lo:hi])

            def c_tile(b, h):
                t = c_sb[b][h]
                return t[1] if needs_cast else t

            # order: w0 + first chunk of batch 0 first so the PE can start,
            # then the rest of the weights, then remaining signal chunks.
            load_w(0)
            load_c(0, [0])
            for k in range(1, KT):
                load_w(k)
            load_c(0, [1, 2, 3])
            load_c(1, [0, 1, 2, 3])

            def rhs(b, k, n):
                off = n * 512 + (k >> 1)
                return c_tile(b, k & 1)[:, off : off + 512]

            for b in range(BPC):
                for n in range(NT):
                    for q in range(QT):
                        ps_re = ps.tile([128, 512], f32, name=f"psre{b}{n}{q}", tag="psre")
                        ps_im = ps.tile([128, 512], f32, name=f"psim{b}{n}{q}", tag="psim")
                        for k in range(KT):
                            nc.tensor.matmul(
                                ps_re[:],
                                w_sb[k][:, q * 128 : (q + 1) * 128],
                                rhs(b, k, n),
                                start=(k == 0),
                                stop=(k == KT - 1),
                            )
                        for k in range(KT):
                            nc.tensor.matmul(
                                ps_im[:],
                                w_sb[k][:, (q + 4) * 128 : (q + 5) * 128],
                                rhs(b, k, n),
                                start=(k == 0),
                                stop=(k == KT - 1),
                            )
                        sq_re = sqp.tile([128, 512], f32, name=f"sqre{b}{n}{q}", tag="sqre")
                        sq_im = sqp.tile([128, 512], f32, name=f"sqim{b}{n}{q}", tag="sqim")
                        nc.scalar.square(sq_re[:], ps_re[:])
                        nc.scalar.square(sq_im[:], ps_im[:])
                        s = sp.tile([128, 512], f32, name=f"s{b}{n}{q}", tag="s")
                        # s = max(re^2, eps) + im^2  (~= clip(re^2+im^2, eps),
                        # exact whenever re^2+im^2 >= eps)
                        nc.vector.scalar_tensor_tensor(
                            s[:],
                            sq_re[:],
                            1e-12,
                            sq_im[:],
                            op0=mybir.AluOpType.max,
                            op1=mybir.AluOpType.add,
                        )
                        if q == 0:
                            # tile pair 0/4 packs cos_512 into the im slot of
                            # row 0; bin 0 is |re_0| and bin 512 is |re_512|.
                            nc.vector.tensor_scalar_max(s[0:1, :], sq_re[0:1, :], 1e-12)
                            r512 = r512p.tile([1, 512], f16, name=f"r512{b}{n}", tag="r512")
                            nc.vector.tensor_scalar_max(r512[:], sq_im[0:1, :], 1e-12)
                            nc.scalar.sqrt(r512[:], r512[:])
                            nc.gpsimd.dma_start(
                                out[b, F - 1 : F, n * 512 : (n + 1) * 512], r512[:]
                            )
                        st = stp.tile([128, 512], f32, name=f"st{b}{n}{q}", tag="st")
                        nc.scalar.sqrt(st[:], s[:])
                        nc.sync.dma_start(
                            out[b, q * 128 : (q + 1) * 128, n * 512 : (n + 1) * 512],
                            st[:],
                        )
    nc.finalize()
    return nc


def _build_program_fold():
    """K=512 variant: the Hann-windowed DFT rows are (anti)symmetric about
    the frame center, so contracting folded frames

      Fplus[m]  = x[m] + x[1024-m]   (cos rows,  m = 1..511)
      Fminus[m] = x[m] - x[1024-m]   (sin rows)

    halves the tensor-engine work.  Slot m=0 carries zero window weight and
    is repurposed for the self-paired center sample x[512] (weight column
    w2[:, 512]), which also folds bin 512 in as one extra M row.  Folds are
    cheap shifted-slice adds of the C layout and a host-built partition-
    reversed copy D_g[p, c] = xp[256c - 128g - p].
    """
    import concourse.bacc as bacc
    import concourse.mybir as mybir
    import concourse.tile as tile

    f32 = mybir.dt.float32
    f16 = mybir.dt.float16

    nc = bacc.Bacc("TRN2", target_bir_lowering=False, debug=False)
    wp_in = nc.declare_dram_parameter("wp", [4, 128, 513], f16, isOutput=False)
    wm_in = nc.declare_dram_parameter("wm", [4, 128, 512], f16, isOutput=False)
    c_in = nc.declare_dram_parameter("c", [BPC, 2, 128, NCHUNK], f16, isOutput=False)
    d_in = nc.declare_dram_parameter("d", [BPC, 2, 128, NCHUNK], f16, isOutput=False)
    out = nc.declare_dram_parameter("out", [BPC, F, L], f16, isOutput=True)

    CH0 = 516  # first-column chunk so the pipeline can start early

    with tile.TileContext(nc) as tc:
        with (
            tc.tile_pool(name="wtp", bufs=1) as wtp,
            tc.tile_pool(name="cdp", bufs=2) as cdp,
            tc.tile_pool(name="fp", bufs=2) as fp,
            tc.tile_pool(name="pcp", bufs=4, space="PSUM") as pcp,
            tc.tile_pool(name="psp", bufs=3, space="PSUM") as psp,
            tc.tile_pool(name="p512p", bufs=1, space="PSUM") as p512p,
            tc.tile_pool(name="sqp", bufs=3) as sqp,
            tc.tile_pool(name="sp", bufs=3) as sp,
            tc.tile_pool(name="stp", bufs=3) as stp,
            tc.tile_pool(name="r512p", bufs=2) as r512p,
        ):
            cd_sb = [None] * BPC

            def load_cd(b, lo, hi):
                if cd_sb[b] is None:
                    cd_sb[b] = (
                        [
                            cdp.tile([128, NCHUNK], f16, name=f"c{h}", tag=f"c{h}")
                            for h in range(2)
                        ],
                        [
                            cdp.tile([128, NCHUNK], f16, name=f"d{h}", tag=f"d{h}")
                            for h in range(2)
                        ],
                    )
                c_sb, d_sb = cd_sb[b]
                for h in range(2):
                    nc.sync.dma_start(c_sb[h][:, lo:hi], c_in[b, h, :, lo:hi])
                    nc.scalar.dma_start(d_sb[h][:, lo:hi], d_in[b, h, :, lo:hi])

            # DMA ring order: batch-0 first chunk, cos weights, batch-0 rest,
            # sin weights — matches the order the PE consumes them.
            load_cd(0, 0, CH0)

            wp_sb, wm_sb = [], []
            for a in range(4):
                t = wtp.tile([128, 513], f16, name=f"wpa{a}")
                nc.gpsimd.dma_start(t[:], wp_in[a])
                wp_sb.append(t)

            load_cd(0, CH0, NCHUNK)

            for a in range(4):
                t = wtp.tile([128, 512], f16, name=f"wma{a}")
                nc.gpsimd.dma_start(t[:], wm_in[a])
                wm_sb.append(t)

            for b in range(BPC):
                c_sb, d_sb = cd_sb[b]
                fpl = [
                    fp.tile([128, L], f16, name=f"fp{a}", tag=f"fp{a}")
                    for a in range(4)
                ]
                fmi = [
                    fp.tile([128, L], f16, name=f"fm{a}", tag=f"fm{a}")
                    for a in range(4)
                ]

                def fold_cols(lo, hi):
                    # plus folds first (cos matmuls consume them first)
                    for sign in range(2):
                        for a in range(4):
                            g = a & 1
                            ao = a >> 1
                            cs = c_sb[g][:, lo + ao : hi + ao]
                            ds = d_sb[g][:, lo + 4 - ao : hi + 4 - ao]
                            nc.vector.tensor_tensor(
                                (fpl if sign == 0 else fmi)[a][:, lo:hi],
                                cs,
                                ds,
                                op=mybir.AluOpType.add
                                if sign == 0
                                else mybir.AluOpType.subtract,
                            )
                        # slot m=0 of both folds carries the self-paired center
                        # sample x[512]; its weight column is w2[:, 512], which
                        # is nonzero even for sin rows (f32 rounding of the
                        # reference angle leaves ~1e-4 there).
                        nc.vector.tensor_copy(
                            (fpl if sign == 0 else fmi)[0][0:1, lo:hi],
                            c_sb[0][0:1, lo + 2 : hi + 2],
                        )

                fold_cols(0, 512)
                fold_cols(512, L)

                for n in range(NT):
                    nsl = slice(n * 512, (n + 1) * 512)
                    if b + 1 < BPC and n == 0:
                        # batch-1 signal streams in while batch-0 computes
                        load_cd(b + 1, 0, CH0)
                        load_cd(b + 1, CH0, NCHUNK)
                    # interleave cos/sin per q so each pair's magnitude
                    # pipeline starts as early as possible
                    pc_t, ps_t = [], []
                    for q in range(QT):
                        pc = pcp.tile([128, 512], f32, name=f"pc{b}{n}{q}", tag="pc")
                        for a in range(4):
                            nc.tensor.matmul(
                                pc[:],
                                wp_sb[a][:, q * 128 : (q + 1) * 128],
                                fpl[a][:, nsl],
                                start=(a == 0),
                                stop=(a == 3),
                            )
                        pc_t.append(pc)
                        pss = psp.tile([128, 512], f32, name=f"psn{b}{n}{q}", tag="ps")
                        for a in range(4):
                            nc.tensor.matmul(
                                pss[:],
                                wm_sb[a][:, q * 128 : (q + 1) * 128],
                                fmi[a][:, nsl],
                                start=(a == 0),
                                stop=(a == 3),
                            )
                        ps_t.append(pss)
                    p512 = p512p.tile([1, 512], f32, name=f"p512{b}{n}", tag="p512")
                    for a in range(4):
                        nc.tensor.matmul(
                            p512[:],
                            wp_sb[a][:, 512:513],
                            fpl[a][:, nsl],
                            start=(a == 0),
                            stop=(a == 3),
                        )

                    r512 = r512p.tile([1, 512], f16, name=f"r512{b}{n}", tag="r512")
                    nc.scalar.activation(
                        r512[:], p512[:], mybir.ActivationFunctionType.Abs
                    )
                    nc.vector.tensor_scalar_max(r512[:], r512[:], 1e-6)
                    nc.gpsimd.dma_start(out[b, F - 1 : F, nsl], r512[:])

                    for q in range(QT):
                        sq_c = sqp.tile([128, 512], f16, name=f"sqc{b}{n}{q}", tag="sqc")
                        sq_s = sqp.tile([128, 512], f16, name=f"sqs{b}{n}{q}", tag="sqs")
                        if q == 3 and not (b == BPC - 1 and n == NT - 1):
                            # relieve the saturated ACT: square the last pair
                            # on DVE via fp16 PSUM copies (fp16 TT runs 2x)
                            cp_c = sqp.tile(
                                [128, 512], f16, name=f"cpc{b}{n}{q}", tag="cpc"
                            )
                            cp_s = sqp.tile(
                                [128, 512], f16, name=f"cps{b}{n}{q}", tag="cps"
                            )
                            nc.vector.tensor_copy(cp_c[:], pc_t[q][:])
                            nc.vector.tensor_copy(cp_s[:], ps_t[q][:])
                            nc.vector.tensor_tensor(
                                sq_c[:], cp_c[:], cp_c[:], op=mybir.AluOpType.mult
                            )
                            nc.vector.tensor_tensor(
                                sq_s[:], cp_s[:], cp_s[:], op=mybir.AluOpType.mult
                            )
                        else:
                            nc.scalar.square(sq_c[:], pc_t[q][:])
                            nc.scalar.square(sq_s[:], ps_t[q][:])
                        s = sp.tile([128, 512], f16, name=f"s{b}{n}{q}", tag="s")
                        # sin bin-0 row is zero, so row 0 automatically gives
                        # sqrt(max(re0^2, eps)) = mag of bin 0.  fp16 tiles:
                        # the STT runs in the DVE 2x perf mode.
                        nc.vector.tensor_tensor(
                            s[:], sq_c[:], sq_s[:], op=mybir.AluOpType.add
                        )
                        st = stp.tile([128, 512], f16, name=f"st{b}{n}{q}", tag="st")
                        nc.scalar.sqrt(st[:], s[:])
                        nc.sync.dma_start(out[b, q * 128 : (q + 1) * 128, nsl], st[:])
    nc.finalize()
    return nc


def _build_program_dif():
    """Radix-2 decimation-in-frequency variant, K=256, all folds on the host.

    Even bins 2a (a=0..255) are the 512-point DFT of u[n] = xw[n]+xw[n+512];
    odd bins 2a+1 come from v[n] = xw[n]-xw[n+512] against the odd-bin basis.
    Both halves fold again about the frame center (cos rows symmetric, sin
    antisymmetric), giving four K=256 fold vectors per frame:

      P  = win*A + wb*B   (even Re)     A  = x[m]+x[1024-m]   wb = 1-win
      Q  = win*A- + wb*B- (even Im)     A- = x[m]-x[1024-m]
      P' = win*A - wb*B   (odd Re)      B  = x[512-m]+x[512+m]
      Q' = win*A- - wb*B- (odd Im)      B- = x[512+m]-x[512-m]

    All of these are shifted-column sums of per-partition-scaled signal
    layouts, so the HOST builds them directly (same total bytes as the old
    C/D layouts) and the device does zero fold work.  Specials: P[0] = x[512]
    (weight 1 on every cos row), u256 = (x[256]+x[768])/2 enters cos-even
    rows with weight (-1)^a via a K=1 matmul, v256 likewise for sin-odd;
    bin 512 = |sum_m (-1)^m P[m] + u256| via an M=1 matmul into the unused
    a=0 row of the sin-even-q0 PSUM tile.

    Per (batch, 512-frame tile): 23 matmuls of N=512 (vs 36 in 'fold').
    Magnitude: ACT squares the cos PSUM pairs (fp16 out), DVE squares the
    sin pairs (TT psum*psum), DVE adds in fp16 (2x mode), ACT sqrts; POOL_SQ
    of the 4 cos squares ride the gpsimd engine instead.  Output rows are
    written as separate even/odd planes (f16) and interleaved on the host.
    """
    import concourse.bacc as bacc
    import concourse.mybir as mybir
    import concourse.tile as tile

    f32 = mybir.dt.float32
    f16 = mybir.dt.float16
    NT_ = NT
    NW = 16  # weight tiles packed in wall

    nc = bacc.Bacc("TRN2", target_bir_lowering=False, debug=False)
    p_in = nc.declare_dram_parameter("p", [BPC, 2, 128, L], f16, isOutput=False)
    q_in = nc.declare_dram_parameter("q", [BPC, 2, 128, L], f16, isOutput=False)
    pp_in = nc.declare_dram_parameter("pp", [BPC, 2, 128, L], f16, isOutput=False)
    qp_in = nc.declare_dram_parameter("qp", [BPC, 2, 128, L], f16, isOutput=False)
    uv_in = nc.declare_dram_parameter("uv", [BPC, 1, 2 * L], f16, isOutput=False)
    wall_in = nc.declare_dram_parameter("wall", [128, NW * 128 + 1], f16, isOutput=False)
    srow_in = nc.declare_dram_parameter("srow", [1, 129], f16, isOutput=False)
    # outi rows: 0..255 = even-bin mags (a), 256..511 = odd-bin mags,
    # 512 = bin-512 row.  Host interleaves.
    outi = nc.declare_dram_parameter("outi", [BPC, F, L], f16, isOutput=True)

    with tile.TileContext(nc) as tc:
        with (
            tc.tile_pool(name="wp", bufs=1) as wp,
            tc.tile_pool(name="sig", bufs=2) as sigp,
            tc.tile_pool(name="ps", bufs=7, space="PSUM") as psp,
            tc.tile_pool(name="p512p", bufs=1, space="PSUM") as p512p,
            tc.tile_pool(name="sqp", bufs=10) as sqp,
            tc.tile_pool(name="sp", bufs=3) as sp,
            tc.tile_pool(name="stp", bufs=3) as stp,
            tc.tile_pool(name="rp", bufs=2) as rp,
        ):
            sig_sb = []
            for b in range(BPC):
                sig_sb.append({
                    "P": [sigp.tile([128, L], f16, name=f"P{b}{h}", tag=f"P{h}") for h in range(2)],
                    "Q": [sigp.tile([128, L], f16, name=f"Q{b}{h}", tag=f"Q{h}") for h in range(2)],
                    "Pp": [sigp.tile([128, L], f16, name=f"Pp{b}{h}", tag=f"Pp{h}") for h in range(2)],
                    "Qp": [sigp.tile([128, L], f16, name=f"Qp{b}{h}", tag=f"Qp{h}") for h in range(2)],
                    "uv": sigp.tile([1, 2 * L], f16, name=f"uv{b}", tag="uv"),
                })

            wall = wp.tile([128, NW * 128 + 1], f16, name="wall")
            srow = wp.tile([1, 129], f16, name="srow")
            # Three parallel DMA queues (SP-HW, ACT-HW, gpsimd-SW), each FIFO.
            # Order by first use: E-pair tensors (P, Q) for both batches lead,
            # O-pair tensors (Pp, Qp) trail behind the first E computes.
            nc.sync.dma_start(wall[:, 0:512], wall_in[:, 0:512])
            nc.sync.dma_start(srow[:], srow_in[:])
            for h in range(2):
                nc.sync.dma_start(sig_sb[0]["P"][h][:], p_in[0, h])
                nc.scalar.dma_start(sig_sb[0]["Q"][h][:], q_in[0, h])
                nc.gpsimd.dma_start(sig_sb[0]["Pp"][h][:], pp_in[0, h])
            nc.gpsimd.dma_start(sig_sb[0]["uv"][:], uv_in[0])
            nc.sync.dma_start(wall[:, 512:], wall_in[:, 512:])
            for h in range(2):
                nc.gpsimd.dma_start(sig_sb[0]["Qp"][h][:], qp_in[0, h])
            for h in range(2):
                nc.sync.dma_start(sig_sb[1]["P"][h][:], p_in[1, h])
                nc.scalar.dma_start(sig_sb[1]["Q"][h][:], q_in[1, h])
            nc.gpsimd.dma_start(sig_sb[1]["uv"][:], uv_in[1])
            for h in range(2):
                nc.scalar.dma_start(sig_sb[1]["Pp"][h][:], pp_in[1, h])
                nc.gpsimd.dma_start(sig_sb[1]["Qp"][h][:], qp_in[1, h])

            def W(i):
                return wall[:, 128 * i : 128 * (i + 1)]

            # wall tile order: wce(q,h), wse(q,h), wco(q,h), wso(q,h); col 2048
            # is the (-1)^p bin-512 column; srow = [(-1)^j (128), one]
            WCE = lambda qq, h: W(0 + 2 * qq + h)
            WSE = lambda qq, h: W(4 + 2 * qq + h)
            WCO = lambda qq, h: W(8 + 2 * qq + h)
            WSO = lambda qq, h: W(12 + 2 * qq + h)
            W512 = wall[:, NW * 128 : NW * 128 + 1]
            SGN = srow[0:1, 0:128]
            ONE = srow[0:1, 128:129]

            rwide = {b: rp.tile([1, L], f16, name=f"rw{b}", tag="rw")
                     for b in range(BPC)}

            for b in range(BPC):
                for n in range(NT_):
                    sig = sig_sb[b]
                    Ph = sig["P"]
                    Qh = sig["Q"]
                    Pph = sig["Pp"]
                    Qph = sig["Qp"]
                    uv = sig["uv"]
                    nsl = slice(n * 512, (n + 1) * 512)
                    usl = slice(n * 512, (n + 1) * 512)

                    # ---- matmuls: 4 (pc, ps) psum pairs ----
                    def mm_cos_even(qq):
                        pc = psp.tile([128, 512], f32, name=f"pcE{qq}_{b}{n}", tag="ps")
                        for h in range(2):
                            nc.tensor.matmul(pc[:], WCE(qq, h), Ph[h][:, nsl],
                                             start=(h == 0), stop=False)
                        nc.tensor.matmul(pc[:], SGN, uv[0:1, usl], start=False, stop=True)
                        return pc

                    def mm_sin_even(qq):
                        # q=0 row 0 has all-zero weights (sin a=0), so the E0
                        # pair's row 0 gives sqrt(re0^2) = bin 0 for free
                        ps = psp.tile([128, 512], f32, name=f"psE{qq}_{b}{n}", tag="ps")
                        for h in range(2):
                            nc.tensor.matmul(ps[:], WSE(qq, h), Qh[h][:, nsl],
                                             start=(h == 0), stop=(h == 1))
                        return ps

                    def mm_bin512():
                        # bin 512 = |sum_m (-1)^m P[m] + u256|
                        p5 = p512p.tile([1, 512], f32, name=f"p512_{b}{n}", tag="p512")
                        for h in range(2):
                            nc.tensor.matmul(p5[:], W512, Ph[h][:, nsl],
                                             start=(h == 0), stop=False)
                        nc.tensor.matmul(p5[:], ONE, uv[0:1, usl], start=False, stop=True)
                        return p5

                    def mm_cos_odd(qq):
                        pc = psp.tile([128, 512], f32, name=f"pcO{qq}_{b}{n}", tag="ps")
                        for h in range(2):
                            nc.tensor.matmul(pc[:], WCO(qq, h), Pph[h][:, nsl],
                                             start=(h == 0), stop=(h == 1))
                        return pc

                    def mm_sin_odd(qq):
                        ps = psp.tile([128, 512], f32, name=f"psO{qq}_{b}{n}", tag="ps")
                        for h in range(2):
                            nc.tensor.matmul(ps[:], WSO(qq, h), Qph[h][:, nsl],
                                             start=(h == 0), stop=(h == 1))
                        return ps

                    # ---- magnitude ----
                    # PSUM reads: ACT any op, DVE max one PSUM operand, Pool
                    # none (verified against the BIR verifier).  ACT squares
                    # the cos tiles (plus alternate sin tiles) straight off
                    # PSUM; remaining sin tiles drain via DVE cast + fp16
                    # self-mult.  All four adds land on Pool (SBUF fp16).
                    # One [128,2048] sqrt per (b,n); its st tile maps to outi
                    # rows 0:512 as four 128-row blocks in a single DMA.
                    def sqtile(tag, p_, act_sq):
                        sq = sqp.tile([128, 512], f16, name=f"sq{tag}{b}{n}", tag="sq")
                        if act_sq:
                            nc.scalar.square(sq[:], p_[:])
                        else:
                            ca = sqp.tile([128, 512], f16, name=f"ca{tag}{b}{n}", tag="ca")
                            nc.vector.tensor_copy(ca[:], p_[:])
                            nc.vector.tensor_tensor(sq[:], ca[:], ca[:],
                                                    op=mybir.AluOpType.mult)
                        return sq

                    def magpair(tag, pc, ps, s, half, act_sin, pool_add):
                        sqc = sqtile("c" + tag, pc, True)
                        sqs = sqtile("s" + tag, ps, act_sin)
                        eng = nc.gpsimd if pool_add else nc.vector
                        eng.tensor_tensor(s[:, 512 * half : 512 * half + 512],
                                          sqc[:], sqs[:], op=mybir.AluOpType.add)

                    s4 = sp.tile([128, 2048], f16, name=f"s4{b}{n}", tag="s")
                    act_sin = ((b * NT_ + n) % 2) == 0

                    pcE0 = mm_cos_even(0)
                    psE0 = mm_sin_even(0)
                    magpair("E0", pcE0, psE0, s4, 0, act_sin=act_sin, pool_add=True)
                    p5 = mm_bin512()
                    nc.scalar.activation(rwide[b][0:1, nsl], p5[:],
                                         mybir.ActivationFunctionType.Abs)
                    pcE1 = mm_cos_even(1)
                    psE1 = mm_sin_even(1)
                    magpair("E1", pcE1, psE1, s4, 1, act_sin=False, pool_add=True)
                    pcO0 = mm_cos_odd(0)
                    psO0 = mm_sin_odd(0)
                    magpair("O0", pcO0, psO0, s4, 2, act_sin=False, pool_add=True)
                    pcO1 = mm_cos_odd(1)
                    psO1 = mm_sin_odd(1)
                    magpair("O1", pcO1, psO1, s4, 3, act_sin=False, pool_add=True)
                    st4 = stp.tile([128, 2048], f16, name=f"st4{b}{n}", tag="st")
                    nc.scalar.sqrt(st4[:], s4[:])
                    if MERGE_OUT:
                        dst = outi[b, 0:512, nsl].rearrange("(k p) t -> p k t", p=128)
                        nc.sync.dma_start(dst, st4[:].rearrange("p (k t) -> p k t", k=4))
                    else:
                        for k in range(4):
                            nc.sync.dma_start(outi[b, 128 * k : 128 * (k + 1), nsl],
                                              st4[:, 512 * k : 512 * (k + 1)])
            for b in range(BPC):
                nc.sync.dma_start(outi[b, F - 1 : F, :], rwide[b][:])
    nc.finalize()
    return nc


def _get_program():
    key = MODE
    if key not in _PROGRAM_CACHE:
        if MODE == "dif":
            _PROGRAM_CACHE[key] = _build_program_dif()
        elif MODE == "fold":
            _PROGRAM_CACHE[key] = _build_program_fold()
        else:
            _PROGRAM_CACHE[key] = _build_program()
    return _PROGRAM_CACHE[key]


def _make_weight_np():
    n = np.arange(N_FFT, dtype=np.float32)
    k = np.arange(N_FFT // 2 + 1, dtype=np.float32)[:, None]
    ang = (-2.0 * np.pi / N_FFT) * k * n[None, :]
    win = 0.5 * (1.0 - np.cos(2.0 * np.pi * n / N_FFT))
    return np.concatenate([np.cos(ang), np.sin(ang)], axis=0) * win  # [1026, 1024]


def _pack_weight(weight):
    if weight is None:
        w2 = _make_weight_np()
    else:
        w2 = np.asarray(weight, dtype=np.float32).reshape(2 * (N_FFT // 2 + 1), N_FFT)
    # rows: cos 0..511, cos 512, sin 1..511  (sin 0 and sin 512 are zero rows)
    w_eff = np.concatenate([w2[0:512], w2[512:513], w2[514:1025]], axis=0)
    # Wt[k, p, m] = w_eff[m, 128k + p]
    wt = np.ascontiguousarray(
        w_eff.T.reshape(KT, 128, N_FFT), dtype=np.float32
    ).astype(_np_w_dtype())
    return wt


def _frame_layout(xb):
    """[SAMPLES] f32 -> C[2, 128, NCHUNK] with C[h, p, c] = xp[256c + 128h + p]."""
    xp = np.empty(NCHUNK * HOP, dtype=np.float32)
    xp[:CACHE] = 0.0
    xp[CACHE : CACHE + SAMPLES] = xb
    xp[CACHE + SAMPLES :] = 0.0
    return np.ascontiguousarray(xp.reshape(NCHUNK, 2, 128).transpose(1, 2, 0))


def _frame_layout_rev(xb):
    """Partition-reversed copy: D[g, p, c] = xp[256c - 128g - p] (0 if oob)."""
    xp = np.empty(NCHUNK * HOP, dtype=np.float32)
    xp[:CACHE] = 0.0
    xp[CACHE : CACHE + SAMPLES] = xb
    xp[CACHE + SAMPLES :] = 0.0
    c = 256 * np.arange(NCHUNK, dtype=np.int64)[None, None, :]
    g = 128 * np.arange(2, dtype=np.int64)[:, None, None]
    p = np.arange(128, dtype=np.int64)[None, :, None]
    idx = c - g - p
    d = xp[np.clip(idx, 0, None)]
    d[idx < 0] = 0.0
    return np.ascontiguousarray(d)


def _pack_weight_fold(weight):
    if weight is None:
        w2 = _make_weight_np()
    else:
        w2 = np.asarray(weight, dtype=np.float32).reshape(2 * (N_FFT // 2 + 1), N_FFT)
    # fold column j contracts x[j] + x[1024-j] (j = 1..511); slot j=0 carries
    # the center sample x[512], whose weight column is w2[:, 512].
    colmap = np.concatenate([[512], np.arange(1, 512)])
    wplus = w2[0:513][:, colmap]  # cos bins 0..512  [513, 512]
    wminus = w2[513:1025][:, colmap]  # sin bins 0..511 (row 0 zero)  [512, 512]
    wp = np.ascontiguousarray(wplus.T.reshape(4, 128, 513)).astype(np.float16)
    wm = np.ascontiguousarray(wminus.T.reshape(4, 128, 512)).astype(np.float16)
    return wp, wm


def _host_prep_dif(xb):
    """[SAMPLES] f32 -> P, Q, Pp, Qp [2,128,L] f16 + uv row [1, 2L] f16."""
    from numpy.lib.stride_tricks import as_strided

    xp = np.zeros(NCHUNK * HOP, dtype=np.float32)
    xp[CACHE : CACHE + SAMPLES] = xb
    s = xp.strides[0]
    X0 = as_strided(xp, (256, L), (s, 256 * s))          # x[256t + m]
    X1024 = as_strided(xp[1024:], (256, L), (-s, 256 * s))  # x[256t + 1024 - m]
    X512p = as_strided(xp[512:], (256, L), (s, 256 * s))    # x[256t + 512 + m]
    X512m = as_strided(xp[512:], (256, L), (-s, 256 * s))   # x[256t + 512 - m]

    n256 = np.arange(256, dtype=np.float32)
    winm = (0.5 * (1.0 - np.cos(2.0 * np.pi * n256 / N_FFT)))[:, None]
    wbm = 1.0 - winm
    A = X0 + X1024
    Am = X0 - X1024
    B = X512m + X512p
    Bm = X512p - X512m
    wA = winm * A
    wB = wbm * B
    wAm = winm * Am
    wBm = wbm * Bm
    P = wA + wB
    Pp = wA - wB
    Q = wAm + wBm
    Qp = wAm - wBm
    u0 = xp[512 : 512 + HOP * L : HOP]
    P[0, :] = u0
    Pp[0, :] = -u0
    Q[0, :] = 0.0
    r256 = xp[256 : 256 + HOP * L : HOP]
    r768 = xp[768 : 768 + HOP * L : HOP]
    u256 = 0.5 * (r256 + r768)
    v256 = 0.5 * (r256 - r768)
    # Q' uses slots m = 1..256: the Nyquist slot m=256 has natural weight
    # sin(pi(2a+1)/2) = (-1)^a, exactly v256's pattern, so v256 rides free.
    Qp_s = np.vstack([Qp[1:256], v256[None, :]])
    uv = np.concatenate([u256, v256])
    f16 = np.float16
    rs = lambda z: np.ascontiguousarray(z.reshape(2, 128, L), dtype=f16)
    return rs(P), rs(Q), rs(Pp), rs(Qp_s), uv.astype(f16)[None, :]


def _pack_weights_dif():
    """wall [128, 16*128+1] f16 (lhsT tiles) + srow [1, 129] f16."""
    j = np.arange(128, dtype=np.float64)[None, :]
    p = np.arange(128, dtype=np.float64)[:, None]
    tiles = []
    for grp in range(4):  # wce, wse, wco, wso
        for q in (0, 1):
            for h in (0, 1):
                m = 128 * h + p
                if grp == 0:
                    t = np.cos(2 * np.pi * (128 * q + j) * m / 512)
                elif grp == 1:
                    t = np.sin(2 * np.pi * (128 * q + j) * m / 512)  # row a=0 zero
                elif grp == 2:
                    t = np.cos(2 * np.pi * (2 * (128 * q + j) + 1) * m / 1024)
                else:
                    t = np.sin(2 * np.pi * (2 * (128 * q + j) + 1) * (m + 1) / 1024)
                tiles.append(t)
    wall = np.empty((128, 16 * 128 + 1), dtype=np.float64)
    wall[:, : 16 * 128] = np.concatenate(tiles, axis=1)
    wall[:, 16 * 128] = (-1.0) ** np.arange(128)  # (-1)^m column for bin 512
    srow = np.empty((1, 129), dtype=np.float64)
    srow[0, :128] = (-1.0) ** np.arange(128)
    srow[0, 128] = 1.0
    return wall.astype(np.float16), srow.astype(np.float16)


def _in_maps_dif(x):
    wall, srow = _pack_weights_dif()
    maps = []
    for i in range(NCORES):
        P, Q, Pp, Qp, uv = [], [], [], [], []
        for b in range(BPC):
            pb, qb, ppb, qpb, uvb = _host_prep_dif(x[BPC * i + b])
            P.append(pb)
            Q.append(qb)
            Pp.append(ppb)
            Qp.append(qpb)
            uv.append(uvb)
        maps.append(
            {
                "p": np.stack(P),
                "q": np.stack(Q),
                "pp": np.stack(Pp),
                "qp": np.stack(Qp),
                "uv": np.stack(uv),
                "wall": wall,
                "srow": srow,
            }
        )
    return maps


def _assemble_dif(results):
    """Interleave even/odd f16 bin planes from each core into f32 output."""
    out = np.empty((BATCH, F, L), dtype=np.float32)
    for i in range(NCORES):
        outi = results[i]["outi"]
        for b in range(BPC):
            out[BPC * i + b, 0:512:2] = outi[b, 0:256]
            out[BPC * i + b, 1:512:2] = outi[b, 256:512]
            out[BPC * i + b, 512] = outi[b, 512]
    return out


def _in_maps(x, weight):
    if MODE == "dif":
        return _in_maps_dif(x)
    if MODE == "fold":
        wp, wm = _pack_weight_fold(weight)
        maps = []
        for i in range(NCORES):
            c = np.stack([_frame_layout(x[BPC * i + b]) for b in range(BPC)])
            d = np.stack([_frame_layout_rev(x[BPC * i + b]) for b in range(BPC)])
            maps.append(
                {"wp": wp, "wm": wm, "c": c.astype(np.float16), "d": d.astype(np.float16)}
            )
        return maps
    wt = _pack_weight(weight)
    maps = []
    for i in range(NCORES):
        c = np.stack([_frame_layout(x[BPC * i + b]) for b in range(BPC)])
        maps.append({"w": wt, "c": c})
    return maps


def kernel(x, weight=None, **_unused):
    from concourse.bass_utils import run_bass_kernel_spmd

    x = np.asarray(x, dtype=np.float32)
    assert x.shape == (BATCH, SAMPLES), x.shape

    nc = _get_program()
    res = run_bass_kernel_spmd(nc, _in_maps(x, weight), core_ids=list(range(NCORES)))

    if MODE == "dif":
        return _assemble_dif(res.results)
    out = np.empty((BATCH, F, L), dtype=np.float32)
    for i in range(NCORES):
        out[BPC * i : BPC * (i + 1)] = res.results[i]["out"]  # f16 -> f32
    return out



# revision 26
# speedup vs baseline: 1.0090x; 1.0090x over previous
"""Causal STFT kernel for Trainium2 (8 NeuronCores, data-parallel over batch).

Problem: x [16, 524288] f32 -> mag [16, 513, 2048] f32.
  Per batch: causal pad 1023 zeros on the left, frames of 1024 at hop 256
  (2048 frames), multiply by Hann-windowed DFT basis (1026 x 1024), take
  per-bin magnitude sqrt(clip(re^2 + im^2, 1e-12)).

Sharding: batch dim split 2 per core across 8 cores (SPMD, no collectives).

Default device strategy, MODE='fold' (~89.4 us HW time, ~3e-4 rel err;
the magnitude pipeline keeps sq/s tiles in fp16 so the DVE add runs in its
2x perf mode).  MODE='dif' is an experimental radix-2 decimation-in-
frequency variant (21 vs 36 matmuls per tile, host-side folds) that is
engine-balanced at ~45us but currently schedules to ~97us on HW.
  - Host relayouts each padded signal (pure layout, each element stored once
    per view): C_h[p, c] = xp[256c + 128h + p] and a partition-reversed copy
    D_g[p, c] = xp[256c - 128g - p], both fp16.  Frame t window position
    m = 128a + p is then C_{a&1}[p, t + (a>>1)], and position 1024 - m is
    D_{a&1}[p, t + 4 - (a>>1)].
  - The Hann-windowed DFT rows are symmetric (cos) / antisymmetric (sin)
    about the frame center, so DVE folds Fplus = C + D, Fminus = C - D
    halve the tensor-engine contraction to K = 512.  The m=0 fold slot has
    zero window weight and is repurposed for the self-paired center sample
    x[512] whose weight column is w2[:, 512]; this also packs the bin-512
    cos row in as a 513th M column of the cos weights.
  - TensorE (fp16): per (batch, 512-frame tile): 4 cos 128-bin tiles +
    1-row bin-512 tile + 4 sin tiles, each accumulating 4 K-chunks in PSUM.
  - ACT squares PSUM pairs, DVE adds them with the eps clip, ACT takes the
    sqrt, outputs stream out per 128x512 tile.  Bin 0 falls out of the
    all-zero sin_0 row; bin 512 is |re_512| via ACT Abs.
"""

import os
import sys

import numpy as np

for _p in ("/opt/trn_rl_repo",):
    if _p not in sys.path and os.path.isdir(_p):
        sys.path.insert(0, _p)

N_FFT = 1024
HOP = 256
CACHE = N_FFT - 1  # 1023 zeros of causal left pad
BATCH = 16
SAMPLES = HOP * 2048
L = 2048  # frames per batch
F = 513  # output bins per batch
NCORES = 8
BPC = BATCH // NCORES  # batches per core = 2
NCHUNK = (CACHE + SAMPLES + 1) // HOP  # 2052 chunks of 256 after padding
KT = N_FFT // 128  # 8 contraction tiles
NT = L // 512  # 4 frame tiles
QT = 4  # 4 (re, im) pair tiles of 128 bins

# matmul mode: 'dif' (radix-2 decimation-in-frequency, K=256, host folds),
# 'fold' (K=512 via window symmetry, fp16), or direct K=1024
# modes 'f32r' (full-rate fp32-in), 'f16', 'bf16', 'f32' (4x slow)
MODE = os.environ.get("STFT_MM_DTYPE", "fold")
# engine assignment knobs for the dif magnitude stage
POOL_SQ = int(os.environ.get("STFT_POOL_SQ", "2"))  # pair squares on gpsimd (0-4)
MERGE_OUT = os.environ.get("STFT_MERGE_OUT", "1") == "1"  # single out-DMA per (b,n)

_PROGRAM_CACHE = {}


def _mm_dtype(mybir):
    return {
        "f32r": mybir.dt.float32r,
        "f32": mybir.dt.float32,
        "f16": mybir.dt.float16,
        "bf16": mybir.dt.bfloat16,
    }[MODE]


def _np_w_dtype():
    import ml_dtypes

    return {
        "f32r": np.float32,
        "f32": np.float32,
        "f16": np.float16,
        "bf16": ml_dtypes.bfloat16,
    }[MODE]


def _build_program():
    import concourse.bacc as bacc
    import concourse.mybir as mybir
    import concourse.tile as tile

    DT = _mm_dtype(mybir)
    f32 = mybir.dt.float32
    needs_cast = MODE in ("f16", "bf16")

    nc = bacc.Bacc("TRN2", target_bir_lowering=False, debug=False)
    w_in = nc.declare_dram_parameter("w", [KT, 128, 1024], DT, isOutput=False)
    c_in = nc.declare_dram_parameter(
        "c", [BPC, 2, 128, NCHUNK], f32 if needs_cast else DT, isOutput=False
    )
    out = nc.declare_dram_parameter("out", [BPC, F, L], f32, isOutput=True)

    # column chunks for the signal loads: n-tile j only needs cols
    # [512j, 512j+516), so chunked DMA+cast lets matmuls start early.
    CB = [0, 516, 1032, 1548, NCHUNK]

    with tile.TileContext(nc) as tc:
        with (
            tc.tile_pool(name="wp", bufs=1) as wp,
            tc.tile_pool(name="cp", bufs=1) as cp,
            tc.tile_pool(name="castp", bufs=1) as castp,
            tc.tile_pool(name="ps", bufs=3, space="PSUM") as ps,
            tc.tile_pool(name="sqp", bufs=3) as sqp,
            tc.tile_pool(name="sp", bufs=3) as sp,
            tc.tile_pool(name="stp", bufs=3) as stp,
            tc.tile_pool(name="r512p", bufs=2) as r512p,
        ):
            w_sb = [None] * KT

            def load_w(k):
                wt = wp.tile([128, 1024], DT, name=f"w{k}")
                nc.sync.dma_start(wt[:], w_in[k])
                w_sb[k] = wt

            c_sb = [[None, None] for _ in range(BPC)]

            def load_c(b, chunks):
                for h in range(2):
                    if c_sb[b][h] is None:
                        c_sb[b][h] = cp.tile(
                            [128, NCHUNK], f32 if needs_cast else DT, name=f"c{b}{h}"
                        )
                        if needs_cast:
                            cast = castp.tile([128, NCHUNK], DT, name=f"cc{b}{h}")
                            c_sb[b][h] = (c_sb[b][h], cast)
                for j in chunks:
                    lo, hi = CB[j], CB[j + 1]
                    for h in range(2):
                        t = c_sb[b][h]
                        if needs_cast:
                            raw, cast = t
                            nc.sync.dma_start(raw[:, lo:hi], c_in[b, h, :, lo:hi])
                            nc.vector.tensor_copy(cast[:, lo:hi], raw[:, lo:hi])
                        else:
                            nc.sync.dma_start(t[:, lo:hi], c_in[b, h, :, lo:hi])

            def c_tile(b, h):
                t = c_sb[b][h]
                return t[1] if needs_cast else t

            # order: w0 + first chunk of batch 0 first so the PE can start,
            # then the rest of the weights, then remaining signal chunks.
            load_w(0)
            load_c(0, [0])
            for k in range(1, KT):
                load_w(k)
            load_c(0, [1, 2, 3])
            load_c(1, [0, 1, 2, 3])

            def rhs(b, k, n):
                off = n * 512 + (k >> 1)
                return c_tile(b, k & 1)[:, off : off + 512]

            for b in range(BPC):
                for n in range(NT):
                    for q in range(QT):
                        ps_re = ps.tile([128, 512], f32, name=f"psre{b}{n}{q}", tag="psre")
                        ps_im = ps.tile([128, 512], f32, name=f"psim{b}{n}{q}", tag="psim")
                        for k in range(KT):
                            nc.tensor.matmul(
                                ps_re[:],
                                w_sb[k][:, q * 128 : (q + 1) * 128],
                                rhs(b, k, n),
                                start=(k == 0),
                                stop=(k == KT - 1),
                            )
                        for k in range(KT):
                            nc.tensor.matmul(
                                ps_im[:],
                                w_sb[k][:, (q + 4) * 128 : (q + 5) * 128],
                                rhs(b, k, n),
                                start=(k == 0),
                                stop=(k == KT - 1),
                            )
                        sq_re = sqp.tile([128, 512], f32, name=f"sqre{b}{n}{q}", tag="sqre")
                        sq_im = sqp.tile([128, 512], f32, name=f"sqim{b}{n}{q}", tag="sqim")
                        nc.scalar.square(sq_re[:], ps_re[:])
                        nc.scalar.square(sq_im[:], ps_im[:])
                        s = sp.tile([128, 512], f32, name=f"s{b}{n}{q}", tag="s")
                        # s = max(re^2, eps) + im^2  (~= clip(re^2+im^2, eps),
                        # exact whenever re^2+im^2 >= eps)
                        nc.vector.scalar_tensor_tensor(
                            s[:],
                            sq_re[:],
                            1e-12,
                            sq_im[:],
                            op0=mybir.AluOpType.max,
                            op1=mybir.AluOpType.add,
                        )
                        if q == 0:
                            # tile pair 0/4 packs cos_512 into the im slot of
                            # row 0; bin 0 is |re_0| and bin 512 is |re_512|.
                            nc.vector.tensor_scalar_max(s[0:1, :], sq_re[0:1, :], 1e-12)
                            r512 = r512p.tile([1, 512], f16, name=f"r512{b}{n}", tag="r512")
                            nc.vector.tensor_scalar_max(r512[:], sq_im[0:1, :], 1e-12)
                            nc.scalar.sqrt(r512[:], r512[:])
                            nc.gpsimd.dma_start(
                                out[b, F - 1 : F, n * 512 : (n + 1) * 512], r512[:]
                            )
                        st = stp.tile([128, 512], f32, name=f"st{b}{n}{q}", tag="st")
                        nc.scalar.sqrt(st[:], s[:])
                        nc.sync.dma_start(
                            out[b, q * 128 : (q + 1) * 128, n * 512 : (n + 1) * 512],
                            st[:],
                        )
    nc.finalize()
    return nc


def _build_program_fold():
    """K=512 variant: the Hann-windowed DFT rows are (anti)symmetric about
    the frame center, so contracting folded frames

      Fplus[m]  = x[m] + x[1024-m]   (cos rows,  m = 1..511)
      Fminus[m] = x[m] - x[1024-m]   (sin rows)

    halves the tensor-engine work.  Slot m=0 carries zero window weight and
    is repurposed for the self-paired center sample x[512] (weight column
    w2[:, 512]), which also folds bin 512 in as one extra M row.  Folds are
    cheap shifted-slice adds of the C layout and a host-built partition-
    reversed copy D_g[p, c] = xp[256c - 128g - p].
    """
    import concourse.bacc as bacc
    import concourse.mybir as mybir
    import concourse.tile as tile

    f32 = mybir.dt.float32
    f16 = mybir.dt.float16

    nc = bacc.Bacc("TRN2", target_bir_lowering=False, debug=False)
    wp_in = nc.declare_dram_parameter("wp", [4, 128, 513], f16, isOutput=False)
    wm_in = nc.declare_dram_parameter("wm", [4, 128, 512], f16, isOutput=False)
    c_in = nc.declare_dram_parameter("c", [BPC, 2, 128, NCHUNK], f16, isOutput=False)
    d_in = nc.declare_dram_parameter("d", [BPC, 2, 128, NCHUNK], f16, isOutput=False)
    out = nc.declare_dram_parameter("out", [BPC, F, L], f16, isOutput=True)

    CH0 = 516  # first-column chunk so the pipeline can start early

    with tile.TileContext(nc) as tc:
        with (
            tc.tile_pool(name="wtp", bufs=1) as wtp,
            tc.tile_pool(name="cdp", bufs=2) as cdp,
            tc.tile_pool(name="fp", bufs=2) as fp,
            tc.tile_pool(name="pcp", bufs=4, space="PSUM") as pcp,
            tc.tile_pool(name="psp", bufs=3, space="PSUM") as psp,
            tc.tile_pool(name="p512p", bufs=1, space="PSUM") as p512p,
            tc.tile_pool(name="sqp", bufs=3) as sqp,
            tc.tile_pool(name="sp", bufs=3) as sp,
            tc.tile_pool(name="stp", bufs=3) as stp,
            tc.tile_pool(name="r512p", bufs=2) as r512p,
        ):
            cd_sb = [None] * BPC

            def load_cd(b, lo, hi):
                if cd_sb[b] is None:
                    cd_sb[b] = (
                        [
                            cdp.tile([128, NCHUNK], f16, name=f"c{h}", tag=f"c{h}")
                            for h in range(2)
                        ],
                        [
                            cdp.tile([128, NCHUNK], f16, name=f"d{h}", tag=f"d{h}")
                            for h in range(2)
                        ],
                    )
                c_sb, d_sb = cd_sb[b]
                for h in range(2):
                    nc.sync.dma_start(c_sb[h][:, lo:hi], c_in[b, h, :, lo:hi])
                    nc.gpsimd.dma_start(d_sb[h][:, lo:hi], d_in[b, h, :, lo:hi])

            # DMA ring order: batch-0 first chunk, cos weights, batch-0 rest,
            # sin weights — matches the order the PE consumes them.
            load_cd(0, 0, CH0)

            wp_sb, wm_sb = [], []
            for a in range(4):
                t = wtp.tile([128, 513], f16, name=f"wpa{a}")
                nc.gpsimd.dma_start(t[:], wp_in[a])
                wp_sb.append(t)

            load_cd(0, CH0, NCHUNK)

            for a in range(4):
                t = wtp.tile([128, 512], f16, name=f"wma{a}")
                nc.gpsimd.dma_start(t[:], wm_in[a])
                wm_sb.append(t)

            for b in range(BPC):
                c_sb, d_sb = cd_sb[b]
                fpl = [
                    fp.tile([128, L], f16, name=f"fp{a}", tag=f"fp{a}")
                    for a in range(4)
                ]
                fmi = [
                    fp.tile([128, L], f16, name=f"fm{a}", tag=f"fm{a}")
                    for a in range(4)
                ]

                def fold_cols(lo, hi):
                    # plus folds first (cos matmuls consume them first)
                    for sign in range(2):
                        for a in range(4):
                            g = a & 1
                            ao = a >> 1
                            cs = c_sb[g][:, lo + ao : hi + ao]
                            ds = d_sb[g][:, lo + 4 - ao : hi + 4 - ao]
                            nc.vector.tensor_tensor(
                                (fpl if sign == 0 else fmi)[a][:, lo:hi],
                                cs,
                                ds,
                                op=mybir.AluOpType.add
                                if sign == 0
                                else mybir.AluOpType.subtract,
                            )
                        # slot m=0 of both folds carries the self-paired center
                        # sample x[512]; its weight column is w2[:, 512], which
                        # is nonzero even for sin rows (f32 rounding of the
                        # reference angle leaves ~1e-4 there).
                        nc.vector.tensor_copy(
                            (fpl if sign == 0 else fmi)[0][0:1, lo:hi],
                            c_sb[0][0:1, lo + 2 : hi + 2],
                        )

                fold_cols(0, 512)
                fold_cols(512, L)

                for n in range(NT):
                    nsl = slice(n * 512, (n + 1) * 512)
                    if b + 1 < BPC and n == 0:
                        # batch-1 signal streams in while batch-0 computes
                        load_cd(b + 1, 0, CH0)
                        load_cd(b + 1, CH0, NCHUNK)
                    # interleave cos/sin per q so each pair's magnitude
                    # pipeline starts as early as possible
                    pc_t, ps_t = [], []
                    for q in range(QT):
                        pc = pcp.tile([128, 512], f32, name=f"pc{b}{n}{q}", tag="pc")
                        for a in range(4):
                            nc.tensor.matmul(
                                pc[:],
                                wp_sb[a][:, q * 128 : (q + 1) * 128],
                                fpl[a][:, nsl],
                                start=(a == 0),
                                stop=(a == 3),
                            )
                        pc_t.append(pc)
                        pss = psp.tile([128, 512], f32, name=f"psn{b}{n}{q}", tag="ps")
                        for a in range(4):
                            nc.tensor.matmul(
                                pss[:],
                                wm_sb[a][:, q * 128 : (q + 1) * 128],
                                fmi[a][:, nsl],
                                start=(a == 0),
                                stop=(a == 3),
                            )
                        ps_t.append(pss)
                    p512 = p512p.tile([1, 512], f32, name=f"p512{b}{n}", tag="p512")
                    for a in range(4):
                        nc.tensor.matmul(
                            p512[:],
                            wp_sb[a][:, 512:513],
                            fpl[a][:, nsl],
                            start=(a == 0),
                            stop=(a == 3),
                        )

                    r512 = r512p.tile([1, 512], f16, name=f"r512{b}{n}", tag="r512")
                    nc.scalar.activation(
                        r512[:], p512[:], mybir.ActivationFunctionType.Abs
                    )
                    nc.vector.tensor_scalar_max(r512[:], r512[:], 1e-6)
                    nc.gpsimd.dma_start(out[b, F - 1 : F, nsl], r512[:])

                    for q in range(QT):
                        sq_c = sqp.tile([128, 512], f16, name=f"sqc{b}{n}{q}", tag="sqc")
                        sq_s = sqp.tile([128, 512], f16, name=f"sqs{b}{n}{q}", tag="sqs")
                        if q == 3 and not (b == BPC - 1 and n == NT - 1):
                            # relieve the saturated ACT: square the last pair
                            # on DVE via fp16 PSUM copies (fp16 TT runs 2x)
                            cp_c = sqp.tile(
                                [128, 512], f16, name=f"cpc{b}{n}{q}", tag="cpc"
                            )
                            cp_s = sqp.tile(
                                [128, 512], f16, name=f"cps{b}{n}{q}", tag="cps"
                            )
                            nc.vector.tensor_copy(cp_c[:], pc_t[q][:])
                            nc.vector.tensor_copy(cp_s[:], ps_t[q][:])
                            nc.vector.tensor_tensor(
                                sq_c[:], cp_c[:], cp_c[:], op=mybir.AluOpType.mult
                            )
                            nc.vector.tensor_tensor(
                                sq_s[:], cp_s[:], cp_s[:], op=mybir.AluOpType.mult
                            )
                        else:
                            nc.scalar.square(sq_c[:], pc_t[q][:])
                            nc.scalar.square(sq_s[:], ps_t[q][:])
                        s = sp.tile([128, 512], f16, name=f"s{b}{n}{q}", tag="s")
                        # sin bin-0 row is zero, so row 0 automatically gives
                        # sqrt(max(re0^2, eps)) = mag of bin 0.  fp16 tiles:
                        # the STT runs in the DVE 2x perf mode.
                        nc.vector.tensor_tensor(
                            s[:], sq_c[:], sq_s[:], op=mybir.AluOpType.add
                        )
                        st = stp.tile([128, 512], f16, name=f"st{b}{n}{q}", tag="st")
                        nc.scalar.sqrt(st[:], s[:])
                        nc.sync.dma_start(out[b, q * 128 : (q + 1) * 128, nsl], st[:])
    nc.finalize()
    return nc


def _build_program_dif():
    """Radix-2 decimation-in-frequency variant, K=256, all folds on the host.

    Even bins 2a (a=0..255) are the 512-point DFT of u[n] = xw[n]+xw[n+512];
    odd bins 2a+1 come from v[n] = xw[n]-xw[n+512] against the odd-bin basis.
    Both halves fold again about the frame center (cos rows symmetric, sin
    antisymmetric), giving four K=256 fold vectors per frame:

      P  = win*A + wb*B   (even Re)     A  = x[m]+x[1024-m]   wb = 1-win
      Q  = win*A- + wb*B- (even Im)     A- = x[m]-x[1024-m]
      P' = win*A - wb*B   (odd Re)      B  = x[512-m]+x[512+m]
      Q' = win*A- - wb*B- (odd Im)      B- = x[512+m]-x[512-m]

    All of these are shifted-column sums of per-partition-scaled signal
    layouts, so the HOST builds them directly (same total bytes as the old
    C/D layouts) and the device does zero fold work.  Specials: P[0] = x[512]
    (weight 1 on every cos row), u256 = (x[256]+x[768])/2 enters cos-even
    rows with weight (-1)^a via a K=1 matmul, v256 likewise for sin-odd;
    bin 512 = |sum_m (-1)^m P[m] + u256| via an M=1 matmul into the unused
    a=0 row of the sin-even-q0 PSUM tile.

    Per (batch, 512-frame tile): 23 matmuls of N=512 (vs 36 in 'fold').
    Magnitude: ACT squares the cos PSUM pairs (fp16 out), DVE squares the
    sin pairs (TT psum*psum), DVE adds in fp16 (2x mode), ACT sqrts; POOL_SQ
    of the 4 cos squares ride the gpsimd engine instead.  Output rows are
    written as separate even/odd planes (f16) and interleaved on the host.
    """
    import concourse.bacc as bacc
    import concourse.mybir as mybir
    import concourse.tile as tile

    f32 = mybir.dt.float32
    f16 = mybir.dt.float16
    NT_ = NT
    NW = 16  # weight tiles packed in wall

    nc = bacc.Bacc("TRN2", target_bir_lowering=False, debug=False)
    p_in = nc.declare_dram_parameter("p", [BPC, 2, 128, L], f16, isOutput=False)
    q_in = nc.declare_dram_parameter("q", [BPC, 2, 128, L], f16, isOutput=False)
    pp_in = nc.declare_dram_parameter("pp", [BPC, 2, 128, L], f16, isOutput=False)
    qp_in = nc.declare_dram_parameter("qp", [BPC, 2, 128, L], f16, isOutput=False)
    uv_in = nc.declare_dram_parameter("uv", [BPC, 1, 2 * L], f16, isOutput=False)
    wall_in = nc.declare_dram_parameter("wall", [128, NW * 128 + 1], f16, isOutput=False)
    srow_in = nc.declare_dram_parameter("srow", [1, 129], f16, isOutput=False)
    # outi rows: 0..255 = even-bin mags (a), 256..511 = odd-bin mags,
    # 512 = bin-512 row.  Host interleaves.
    outi = nc.declare_dram_parameter("outi", [BPC, F, L], f16, isOutput=True)

    with tile.TileContext(nc) as tc:
        with (
            tc.tile_pool(name="wp", bufs=1) as wp,
            tc.tile_pool(name="sig", bufs=2) as sigp,
            tc.tile_pool(name="ps", bufs=7, space="PSUM") as psp,
            tc.tile_pool(name="p512p", bufs=1, space="PSUM") as p512p,
            tc.tile_pool(name="sqp", bufs=10) as sqp,
            tc.tile_pool(name="sp", bufs=3) as sp,
            tc.tile_pool(name="stp", bufs=3) as stp,
            tc.tile_pool(name="rp", bufs=2) as rp,
        ):
            sig_sb = []
            for b in range(BPC):
                sig_sb.append({
                    "P": [sigp.tile([128, L], f16, name=f"P{b}{h}", tag=f"P{h}") for h in range(2)],
                    "Q": [sigp.tile([128, L], f16, name=f"Q{b}{h}", tag=f"Q{h}") for h in range(2)],
                    "Pp": [sigp.tile([128, L], f16, name=f"Pp{b}{h}", tag=f"Pp{h}") for h in range(2)],
                    "Qp": [sigp.tile([128, L], f16, name=f"Qp{b}{h}", tag=f"Qp{h}") for h in range(2)],
                    "uv": sigp.tile([1, 2 * L], f16, name=f"uv{b}", tag="uv"),
                })

            wall = wp.tile([128, NW * 128 + 1], f16, name="wall")
            srow = wp.tile([1, 129], f16, name="srow")
            # Three parallel DMA queues (SP-HW, ACT-HW, gpsimd-SW), each FIFO.
            # Order by first use: E-pair tensors (P, Q) for both batches lead,
            # O-pair tensors (Pp, Qp) trail behind the first E computes.
            nc.sync.dma_start(wall[:, 0:512], wall_in[:, 0:512])
            nc.sync.dma_start(srow[:], srow_in[:])
            for h in range(2):
                nc.sync.dma_start(sig_sb[0]["P"][h][:], p_in[0, h])
                nc.scalar.dma_start(sig_sb[0]["Q"][h][:], q_in[0, h])
                nc.gpsimd.dma_start(sig_sb[0]["Pp"][h][:], pp_in[0, h])
            nc.gpsimd.dma_start(sig_sb[0]["uv"][:], uv_in[0])
            nc.sync.dma_start(wall[:, 512:], wall_in[:, 512:])
            for h in range(2):
                nc.gpsimd.dma_start(sig_sb[0]["Qp"][h][:], qp_in[0, h])
            for h in range(2):
                nc.sync.dma_start(sig_sb[1]["P"][h][:], p_in[1, h])
                nc.scalar.dma_start(sig_sb[1]["Q"][h][:], q_in[1, h])
            nc.gpsimd.dma_start(sig_sb[1]["uv"][:], uv_in[1])
            for h in range(2):
                nc.scalar.dma_start(sig_sb[1]["Pp"][h][:], pp_in[1, h])
                nc.gpsimd.dma_start(sig_sb[1]["Qp"][h][:], qp_in[1, h])

            def W(i):
                return wall[:, 128 * i : 128 * (i + 1)]

            # wall tile order: wce(q,h), wse(q,h), wco(q,h), wso(q,h); col 2048
            # is the (-1)^p bin-512 column; srow = [(-1)^j (128), one]
            WCE = lambda qq, h: W(0 + 2 * qq + h)
            WSE = lambda qq, h: W(4 + 2 * qq + h)
            WCO = lambda qq, h: W(8 + 2 * qq + h)
            WSO = lambda qq, h: W(12 + 2 * qq + h)
            W512 = wall[:, NW * 128 : NW * 128 + 1]
            SGN = srow[0:1, 0:128]
            ONE = srow[0:1, 128:129]

            rwide = {b: rp.tile([1, L], f16, name=f"rw{b}", tag="rw")
                     for b in range(BPC)}

            for b in range(BPC):
                for n in range(NT_):
                    sig = sig_sb[b]
                    Ph = sig["P"]
                    Qh = sig["Q"]
                    Pph = sig["Pp"]
                    Qph = sig["Qp"]
                    uv = sig["uv"]
                    nsl = slice(n * 512, (n + 1) * 512)
                    usl = slice(n * 512, (n + 1) * 512)

                    # ---- matmuls: 4 (pc, ps) psum pairs ----
                    def mm_cos_even(qq):
                        pc = psp.tile([128, 512], f32, name=f"pcE{qq}_{b}{n}", tag="ps")
                        for h in range(2):
                            nc.tensor.matmul(pc[:], WCE(qq, h), Ph[h][:, nsl],
                                             start=(h == 0), stop=False)
                        nc.tensor.matmul(pc[:], SGN, uv[0:1, usl], start=False, stop=True)
                        return pc

                    def mm_sin_even(qq):
                        # q=0 row 0 has all-zero weights (sin a=0), so the E0
                        # pair's row 0 gives sqrt(re0^2) = bin 0 for free
                        ps = psp.tile([128, 512], f32, name=f"psE{qq}_{b}{n}", tag="ps")
                        for h in range(2):
                            nc.tensor.matmul(ps[:], WSE(qq, h), Qh[h][:, nsl],
                                             start=(h == 0), stop=(h == 1))
                        return ps

                    def mm_bin512():
                        # bin 512 = |sum_m (-1)^m P[m] + u256|
                        p5 = p512p.tile([1, 512], f32, name=f"p512_{b}{n}", tag="p512")
                        for h in range(2):
                            nc.tensor.matmul(p5[:], W512, Ph[h][:, nsl],
                                             start=(h == 0), stop=False)
                        nc.tensor.matmul(p5[:], ONE, uv[0:1, usl], start=False, stop=True)
                        return p5

                    def mm_cos_odd(qq):
                        pc = psp.tile([128, 512], f32, name=f"pcO{qq}_{b}{n}", tag="ps")
                        for h in range(2):
                            nc.tensor.matmul(pc[:], WCO(qq, h), Pph[h][:, nsl],
                                             start=(h == 0), stop=(h == 1))
                        return pc

                    def mm_sin_odd(qq):
                        ps = psp.tile([128, 512], f32, name=f"psO{qq}_{b}{n}", tag="ps")
                        for h in range(2):
                            nc.tensor.matmul(ps[:], WSO(qq, h), Qph[h][:, nsl],
                                             start=(h == 0), stop=(h == 1))
                        return ps

                    # ---- magnitude ----
                    # PSUM reads: ACT any op, DVE max one PSUM operand, Pool
                    # none (verified against the BIR verifier).  ACT squares
                    # the cos tiles (plus alternate sin tiles) straight off
                    # PSUM; remaining sin tiles drain via DVE cast + fp16
                    # self-mult.  All four adds land on Pool (SBUF fp16).
                    # One [128,2048] sqrt per (b,n); its st tile maps to outi
                    # rows 0:512 as four 128-row blocks in a single DMA.
                    def sqtile(tag, p_, act_sq):
                        sq = sqp.tile([128, 512], f16, name=f"sq{tag}{b}{n}", tag="sq")
                        if act_sq:
                            nc.scalar.square(sq[:], p_[:])
                        else:
                            ca = sqp.tile([128, 512], f16, name=f"ca{tag}{b}{n}", tag="ca")
                            nc.vector.tensor_copy(ca[:], p_[:])
                            nc.vector.tensor_tensor(sq[:], ca[:], ca[:],
                                                    op=mybir.AluOpType.mult)
                        return sq

                    def magpair(tag, pc, ps, s, half, act_sin, pool_add):
                        sqc = sqtile("c" + tag, pc, True)
                        sqs = sqtile("s" + tag, ps, act_sin)
                        eng = nc.gpsimd if pool_add else nc.vector
                        eng.tensor_tensor(s[:, 512 * half : 512 * half + 512],
                                          sqc[:], sqs[:], op=mybir.AluOpType.add)

                    s4 = sp.tile([128, 2048], f16, name=f"s4{b}{n}", tag="s")
                    act_sin = ((b * NT_ + n) % 2) == 0

                    pcE0 = mm_cos_even(0)
                    psE0 = mm_sin_even(0)
                    magpair("E0", pcE0, psE0, s4, 0, act_sin=act_sin, pool_add=True)
                    p5 = mm_bin512()
                    nc.scalar.activation(rwide[b][0:1, nsl], p5[:],
                                         mybir.ActivationFunctionType.Abs)
                    pcE1 = mm_cos_even(1)
                    psE1 = mm_sin_even(1)
                    magpair("E1", pcE1, psE1, s4, 1, act_sin=False, pool_add=True)
                    pcO0 = mm_cos_odd(0)
                    psO0 = mm_sin_odd(0)
                    magpair("O0", pcO0, psO0, s4, 2, act_sin=False, pool_add=True)
                    pcO1 = mm_cos_odd(1)
                    psO1 = mm_sin_odd(1)
                    magpair("O1", pcO1, psO1, s4, 3, act_sin=False, pool_add=True)
                    st4 = stp.tile([128, 2048], f16, name=f"st4{b}{n}", tag="st")
                    nc.scalar.sqrt(st4[:], s4[:])
                    if MERGE_OUT:
                        dst = outi[b, 0:512, nsl].rearrange("(k p) t -> p k t", p=128)
                        nc.sync.dma_start(dst, st4[:].rearrange("p (k t) -> p k t", k=4))
                    else:
                        for k in range(4):
                            nc.sync.dma_start(outi[b, 128 * k : 128 * (k + 1), nsl],
                                              st4[:, 512 * k : 512 * (k + 1)])
            for b in range(BPC):
                nc.sync.dma_start(outi[b, F - 1 : F, :], rwide[b][:])
    nc.finalize()
    return nc


def _get_program():
    key = MODE
    if key not in _PROGRAM_CACHE:
        if MODE == "dif":
            _PROGRAM_CACHE[key] = _build_program_dif()
        elif MODE == "fold":
            _PROGRAM_CACHE[key] = _build_program_fold()
        else:
            _PROGRAM_CACHE[key] = _build_program()
    return _PROGRAM_CACHE[key]


def _make_weight_np():
    n = np.arange(N_FFT, dtype=np.float32)
    k = np.arange(N_FFT // 2 + 1, dtype=np.float32)[:, None]
    ang = (-2.0 * np.pi / N_FFT) * k * n[None, :]
    win = 0.5 * (1.0 - np.cos(2.0 * np.pi * n / N_FFT))
    return np.concatenate([np.cos(ang), np.sin(ang)], axis=0) * win  # [1026, 1024]


def _pack_weight(weight):
    if weight is None:
        w2 = _make_weight_np()
    else:
        w2 = np.asarray(weight, dtype=np.float32).reshape(2 * (N_FFT // 2 + 1), N_FFT)
    # rows: cos 0..511, cos 512, sin 1..511  (sin 0 and sin 512 are zero rows)
    w_eff = np.concatenate([w2[0:512], w2[512:513], w2[514:1025]], axis=0)
    # Wt[k, p, m] = w_eff[m, 128k + p]
    wt = np.ascontiguousarray(
        w_eff.T.reshape(KT, 128, N_FFT), dtype=np.float32
    ).astype(_np_w_dtype())
    return wt


def _frame_layout(xb):
    """[SAMPLES] f32 -> C[2, 128, NCHUNK] with C[h, p, c] = xp[256c + 128h + p]."""
    xp = np.empty(NCHUNK * HOP, dtype=np.float32)
    xp[:CACHE] = 0.0
    xp[CACHE : CACHE + SAMPLES] = xb
    xp[CACHE + SAMPLES :] = 0.0
    return np.ascontiguousarray(xp.reshape(NCHUNK, 2, 128).transpose(1, 2, 0))


def _frame_layout_rev(xb):
    """Partition-reversed copy: D[g, p, c] = xp[256c - 128g - p] (0 if oob)."""
    xp = np.empty(NCHUNK * HOP, dtype=np.float32)
    xp[:CACHE] = 0.0
    xp[CACHE : CACHE + SAMPLES] = xb
    xp[CACHE + SAMPLES :] = 0.0
    c = 256 * np.arange(NCHUNK, dtype=np.int64)[None, None, :]
    g = 128 * np.arange(2, dtype=np.int64)[:, None, None]
    p = np.arange(128, dtype=np.int64)[None, :, None]
    idx = c - g - p
    d = xp[np.clip(idx, 0, None)]
    d[idx < 0] = 0.0
    return np.ascontiguousarray(d)


def _pack_weight_fold(weight):
    if weight is None:
        w2 = _make_weight_np()
    else:
        w2 = np.asarray(weight, dtype=np.float32).reshape(2 * (N_FFT // 2 + 1), N_FFT)
    # fold column j contracts x[j] + x[1024-j] (j = 1..511); slot j=0 carries
    # the center sample x[512], whose weight column is w2[:, 512].
    colmap = np.concatenate([[512], np.arange(1, 512)])
    wplus = w2[0:513][:, colmap]  # cos bins 0..512  [513, 512]
    wminus = w2[513:1025][:, colmap]  # sin bins 0..511 (row 0 zero)  [512, 512]
    wp = np.ascontiguousarray(wplus.T.reshape(4, 128, 513)).astype(np.float16)
    wm = np.ascontiguousarray(wminus.T.reshape(4, 128, 512)).astype(np.float16)
    return wp, wm


def _host_prep_dif(xb):
    """[SAMPLES] f32 -> P, Q, Pp, Qp [2,128,L] f16 + uv row [1, 2L] f16."""
    from numpy.lib.stride_tricks import as_strided

    xp = np.zeros(NCHUNK * HOP, dtype=np.float32)
    xp[CACHE : CACHE + SAMPLES] = xb
    s = xp.strides[0]
    X0 = as_strided(xp, (256, L), (s, 256 * s))          # x[256t + m]
    X1024 = as_strided(xp[1024:], (256, L), (-s, 256 * s))  # x[256t + 1024 - m]
    X512p = as_strided(xp[512:], (256, L), (s, 256 * s))    # x[256t + 512 + m]
    X512m = as_strided(xp[512:], (256, L), (-s, 256 * s))   # x[256t + 512 - m]

    n256 = np.arange(256, dtype=np.float32)
    winm = (0.5 * (1.0 - np.cos(2.0 * np.pi * n256 / N_FFT)))[:, None]
    wbm = 1.0 - winm
    A = X0 + X1024
    Am = X0 - X1024
    B = X512m + X512p
    Bm = X512p - X512m
    wA = winm * A
    wB = wbm * B
    wAm = winm * Am
    wBm = wbm * Bm
    P = wA + wB
    Pp = wA - wB
    Q = wAm + wBm
    Qp = wAm - wBm
    u0 = xp[512 : 512 + HOP * L : HOP]
    P[0, :] = u0
    Pp[0, :] = -u0
    Q[0, :] = 0.0
    r256 = xp[256 : 256 + HOP * L : HOP]
    r768 = xp[768 : 768 + HOP * L : HOP]
    u256 = 0.5 * (r256 + r768)
    v256 = 0.5 * (r256 - r768)
    # Q' uses slots m = 1..256: the Nyquist slot m=256 has natural weight
    # sin(pi(2a+1)/2) = (-1)^a, exactly v256's pattern, so v256 rides free.
    Qp_s = np.vstack([Qp[1:256], v256[None, :]])
    uv = np.concatenate([u256, v256])
    f16 = np.float16
    rs = lambda z: np.ascontiguousarray(z.reshape(2, 128, L), dtype=f16)
    return rs(P), rs(Q), rs(Pp), rs(Qp_s), uv.astype(f16)[None, :]


def _pack_weights_dif():
    """wall [128, 16*128+1] f16 (lhsT tiles) + srow [1, 129] f16."""
    j = np.arange(128, dtype=np.float64)[None, :]
    p = np.arange(128, dtype=np.float64)[:, None]
    tiles = []
    for grp in range(4):  # wce, wse, wco, wso
        for q in (0, 1):
            for h in (0, 1):
                m = 128 * h + p
                if grp == 0:
                    t = np.cos(2 * np.pi * (128 * q + j) * m / 512)
                elif grp == 1:
                    t = np.sin(2 * np.pi * (128 * q + j) * m / 512)  # row a=0 zero
                elif grp == 2:
                    t = np.cos(2 * np.pi * (2 * (128 * q + j) + 1) * m / 1024)
                else:
                    t = np.sin(2 * np.pi * (2 * (128 * q + j) + 1) * (m + 1) / 1024)
                tiles.append(t)
    wall = np.empty((128, 16 * 128 + 1), dtype=np.float64)
    wall[:, : 16 * 128] = np.concatenate(tiles, axis=1)
    wall[:, 16 * 128] = (-1.0) ** np.arange(128)  # (-1)^m column for bin 512
    srow = np.empty((1, 129), dtype=np.float64)
    srow[0, :128] = (-1.0) ** np.arange(128)
    srow[0, 128] = 1.0
    return wall.astype(np.float16), srow.astype(np.float16)


def _in_maps_dif(x):
    wall, srow = _pack_weights_dif()
    maps = []
    for i in range(NCORES):
        P, Q, Pp, Qp, uv = [], [], [], [], []
        for b in range(BPC):
            pb, qb, ppb, qpb, uvb = _host_prep_dif(x[BPC * i + b])
            P.append(pb)
            Q.append(qb)
            Pp.append(ppb)
            Qp.append(qpb)
            uv.append(uvb)
        maps.append(
            {
                "p": np.stack(P),
                "q": np.stack(Q),
                "pp": np.stack(Pp),
                "qp": np.stack(Qp),
                "uv": np.stack(uv),
                "wall": wall,
                "srow": srow,
            }
        )
    return maps


def _assemble_dif(results):
    """Interleave even/odd f16 bin planes from each core into f32 output."""
    out = np.empty((BATCH, F, L), dtype=np.float32)
    for i in range(NCORES):
        outi = results[i]["outi"]
        for b in range(BPC):
            out[BPC * i + b, 0:512:2] = outi[b, 0:256]
            out[BPC * i + b, 1:512:2] = outi[b, 256:512]
            out[BPC * i + b, 512] = outi[b, 512]
    return out


def _in_maps(x, weight):
    if MODE == "dif":
        return _in_maps_dif(x)
    if MODE == "fold":
        wp, wm = _pack_weight_fold(weight)
        maps = []
        for i in range(NCORES):
            c = np.stack([_frame_layout(x[BPC * i + b]) for b in range(BPC)])
            d = np.stack([_frame_layout_rev(x[BPC * i + b]) for b in range(BPC)])
            maps.append(
                {"wp": wp, "wm": wm, "c": c.astype(np.float16), "d": d.astype(np.float16)}
            )
        return maps
    wt = _pack_weight(weight)
    maps = []
    for i in range(NCORES):
        c = np.stack([_frame_layout(x[BPC * i + b]) for b in range(BPC)])
        maps.append({"w": wt, "c": c})
    return maps


def kernel(x, weight=None, **_unused):
    from concourse.bass_utils import run_bass_kernel_spmd

    x = np.asarray(x, dtype=np.float32)
    assert x.shape == (BATCH, SAMPLES), x.shape

    nc = _get_program()
    res = run_bass_kernel_spmd(nc, _in_maps(x, weight), core_ids=list(range(NCORES)))

    if MODE == "dif":
        return _assemble_dif(res.results)
    out = np.empty((BATCH, F, L), dtype=np.float32)
    for i in range(NCORES):
        out[BPC * i : BPC * (i + 1)] = res.results[i]["out"]  # f16 -> f32
    return out



# revision 27
# speedup vs baseline: 1.0584x; 1.0489x over previous
"""Causal STFT kernel for Trainium2 (8 NeuronCores, data-parallel over batch).

Problem: x [16, 524288] f32 -> mag [16, 513, 2048] f32.
  Per batch: causal pad 1023 zeros on the left, frames of 1024 at hop 256
  (2048 frames), multiply by Hann-windowed DFT basis (1026 x 1024), take
  per-bin magnitude sqrt(clip(re^2 + im^2, 1e-12)).

Sharding: batch dim split 2 per core across 8 cores (SPMD, no collectives).

Default device strategy, MODE='fold' (~89.4 us HW time, ~3e-4 rel err;
the magnitude pipeline keeps sq/s tiles in fp16 so the DVE add runs in its
2x perf mode).  MODE='dif' is an experimental radix-2 decimation-in-
frequency variant (21 vs 36 matmuls per tile, host-side folds) that is
engine-balanced at ~45us but currently schedules to ~97us on HW.
  - Host relayouts each padded signal (pure layout, each element stored once
    per view): C_h[p, c] = xp[256c + 128h + p] and a partition-reversed copy
    D_g[p, c] = xp[256c - 128g - p], both fp16.  Frame t window position
    m = 128a + p is then C_{a&1}[p, t + (a>>1)], and position 1024 - m is
    D_{a&1}[p, t + 4 - (a>>1)].
  - The Hann-windowed DFT rows are symmetric (cos) / antisymmetric (sin)
    about the frame center, so DVE folds Fplus = C + D, Fminus = C - D
    halve the tensor-engine contraction to K = 512.  The m=0 fold slot has
    zero window weight and is repurposed for the self-paired center sample
    x[512] whose weight column is w2[:, 512]; this also packs the bin-512
    cos row in as a 513th M column of the cos weights.
  - TensorE (fp16): per (batch, 512-frame tile): 4 cos 128-bin tiles +
    1-row bin-512 tile + 4 sin tiles, each accumulating 4 K-chunks in PSUM.
  - ACT squares PSUM pairs, DVE adds them with the eps clip, ACT takes the
    sqrt, outputs stream out per 128x512 tile.  Bin 0 falls out of the
    all-zero sin_0 row; bin 512 is |re_512| via ACT Abs.
"""

import os
import sys

import numpy as np

for _p in ("/opt/trn_rl_repo",):
    if _p not in sys.path and os.path.isdir(_p):
        sys.path.insert(0, _p)

N_FFT = 1024
HOP = 256
CACHE = N_FFT - 1  # 1023 zeros of causal left pad
BATCH = 16
SAMPLES = HOP * 2048
L = 2048  # frames per batch
F = 513  # output bins per batch
NCORES = 8
BPC = BATCH // NCORES  # batches per core = 2
NCHUNK = (CACHE + SAMPLES + 1) // HOP  # 2052 chunks of 256 after padding
KT = N_FFT // 128  # 8 contraction tiles
NT = L // 512  # 4 frame tiles
QT = 4  # 4 (re, im) pair tiles of 128 bins

# matmul mode: 'dif' (radix-2 decimation-in-frequency, K=256, host folds),
# 'fold' (K=512 via window symmetry, fp16), or direct K=1024
# modes 'f32r' (full-rate fp32-in), 'f16', 'bf16', 'f32' (4x slow)
MODE = os.environ.get("STFT_MM_DTYPE", "fold")
# engine assignment knobs for the dif magnitude stage
POOL_SQ = int(os.environ.get("STFT_POOL_SQ", "2"))  # pair squares on gpsimd (0-4)
MERGE_OUT = os.environ.get("STFT_MERGE_OUT", "1") == "1"  # single out-DMA per (b,n)

_PROGRAM_CACHE = {}


def _mm_dtype(mybir):
    return {
        "f32r": mybir.dt.float32r,
        "f32": mybir.dt.float32,
        "f16": mybir.dt.float16,
        "bf16": mybir.dt.bfloat16,
    }[MODE]


def _np_w_dtype():
    import ml_dtypes

    return {
        "f32r": np.float32,
        "f32": np.float32,
        "f16": np.float16,
        "bf16": ml_dtypes.bfloat16,
    }[MODE]


def _build_program():
    import concourse.bacc as bacc
    import concourse.mybir as mybir
    import concourse.tile as tile

    DT = _mm_dtype(mybir)
    f32 = mybir.dt.float32
    needs_cast = MODE in ("f16", "bf16")

    nc = bacc.Bacc("TRN2", target_bir_lowering=False, debug=False)
    w_in = nc.declare_dram_parameter("w", [KT, 128, 1024], DT, isOutput=False)
    c_in = nc.declare_dram_parameter(
        "c", [BPC, 2, 128, NCHUNK], f32 if needs_cast else DT, isOutput=False
    )
    out = nc.declare_dram_parameter("out", [BPC, F, L], f32, isOutput=True)

    # column chunks for the signal loads: n-tile j only needs cols
    # [512j, 512j+516), so chunked DMA+cast lets matmuls start early.
    CB = [0, 516, 1032, 1548, NCHUNK]

    with tile.TileContext(nc) as tc:
        with (
            tc.tile_pool(name="wp", bufs=1) as wp,
            tc.tile_pool(name="cp", bufs=1) as cp,
            tc.tile_pool(name="castp", bufs=1) as castp,
            tc.tile_pool(name="ps", bufs=3, space="PSUM") as ps,
            tc.tile_pool(name="sqp", bufs=3) as sqp,
            tc.tile_pool(name="sp", bufs=3) as sp,
            tc.tile_pool(name="stp", bufs=3) as stp,
            tc.tile_pool(name="r512p", bufs=2) as r512p,
        ):
            w_sb = [None] * KT

            def load_w(k):
                wt = wp.tile([128, 1024], DT, name=f"w{k}")
                nc.sync.dma_start(wt[:], w_in[k])
                w_sb[k] = wt

            c_sb = [[None, None] for _ in range(BPC)]

            def load_c(b, chunks):
                for h in range(2):
                    if c_sb[b][h] is None:
                        c_sb[b][h] = cp.tile(
                            [128, NCHUNK], f32 if needs_cast else DT, name=f"c{b}{h}"
                        )
                        if needs_cast:
                            cast = castp.tile([128, NCHUNK], DT, name=f"cc{b}{h}")
                            c_sb[b][h] = (c_sb[b][h], cast)
                for j in chunks:
                    lo, hi = CB[j], CB[j + 1]
                    for h in range(2):
                        t = c_sb[b][h]
                        if needs_cast:
                            raw, cast = t
                            nc.sync.dma_start(raw[:, lo:hi], c_in[b, h, :, lo:hi])
                            nc.vector.tensor_copy(cast[:, lo:hi], raw[:, lo:hi])
                        else:
                            nc.sync.dma_start(t[:, lo:hi], c_in[b, h, :, lo:hi])

            def c_tile(b, h):
                t = c_sb[b][h]
                return t[1] if needs_cast else t

            # order: w0 + first chunk of batch 0 first so the PE can start,
            # then the rest of the weights, then remaining signal chunks.
            load_w(0)
            load_c(0, [0])
            for k in range(1, KT):
                load_w(k)
            load_c(0, [1, 2, 3])
            load_c(1, [0, 1, 2, 3])

            def rhs(b, k, n):
                off = n * 512 + (k >> 1)
                return c_tile(b, k & 1)[:, off : off + 512]

            for b in range(BPC):
                for n in range(NT):
                    for q in range(QT):
                        ps_re = ps.tile([128, 512], f32, name=f"psre{b}{n}{q}", tag="psre")
                        ps_im = ps.tile([128, 512], f32, name=f"psim{b}{n}{q}", tag="psim")
                        for k in range(KT):
                            nc.tensor.matmul(
                                ps_re[:],
                                w_sb[k][:, q * 128 : (q + 1) * 128],
                                rhs(b, k, n),
                                start=(k == 0),
                                stop=(k == KT - 1),
                            )
                        for k in range(KT):
                            nc.tensor.matmul(
                                ps_im[:],
                                w_sb[k][:, (q + 4) * 128 : (q + 5) * 128],
                                rhs(b, k, n),
                                start=(k == 0),
                                stop=(k == KT - 1),
                            )
                        sq_re = sqp.tile([128, 512], f32, name=f"sqre{b}{n}{q}", tag="sqre")
                        sq_im = sqp.tile([128, 512], f32, name=f"sqim{b}{n}{q}", tag="sqim")
                        nc.scalar.square(sq_re[:], ps_re[:])
                        nc.scalar.square(sq_im[:], ps_im[:])
                        s = sp.tile([128, 512], f32, name=f"s{b}{n}{q}", tag="s")
                        # s = max(re^2, eps) + im^2  (~= clip(re^2+im^2, eps),
                        # exact whenever re^2+im^2 >= eps)
                        nc.vector.scalar_tensor_tensor(
                            s[:],
                            sq_re[:],
                            1e-12,
                            sq_im[:],
                            op0=mybir.AluOpType.max,
                            op1=mybir.AluOpType.add,
                        )
                        if q == 0:
                            # tile pair 0/4 packs cos_512 into the im slot of
                            # row 0; bin 0 is |re_0| and bin 512 is |re_512|.
                            nc.vector.tensor_scalar_max(s[0:1, :], sq_re[0:1, :], 1e-12)
                            r512 = r512p.tile([1, 512], f16, name=f"r512{b}{n}", tag="r512")
                            nc.vector.tensor_scalar_max(r512[:], sq_im[0:1, :], 1e-12)
                            nc.scalar.sqrt(r512[:], r512[:])
                            nc.gpsimd.dma_start(
                                out[b, F - 1 : F, n * 512 : (n + 1) * 512], r512[:]
                            )
                        st = stp.tile([128, 512], f32, name=f"st{b}{n}{q}", tag="st")
                        nc.scalar.sqrt(st[:], s[:])
                        nc.sync.dma_start(
                            out[b, q * 128 : (q + 1) * 128, n * 512 : (n + 1) * 512],
                            st[:],
                        )
    nc.finalize()
    return nc


def _build_program_fold():
    """K=512 variant: the Hann-windowed DFT rows are (anti)symmetric about
    the frame center, so contracting folded frames

      Fplus[m]  = x[m] + x[1024-m]   (cos rows,  m = 1..511)
      Fminus[m] = x[m] - x[1024-m]   (sin rows)

    halves the tensor-engine work.  Slot m=0 carries zero window weight and
    is repurposed for the self-paired center sample x[512] (weight column
    w2[:, 512]), which also folds bin 512 in as one extra M row.  Folds are
    cheap shifted-slice adds of the C layout and a host-built partition-
    reversed copy D_g[p, c] = xp[256c - 128g - p].
    """
    import concourse.bacc as bacc
    import concourse.mybir as mybir
    import concourse.tile as tile

    f32 = mybir.dt.float32
    f16 = mybir.dt.float16

    nc = bacc.Bacc("TRN2", target_bir_lowering=False, debug=False)
    wp_in = nc.declare_dram_parameter("wp", [4, 128, 513], f16, isOutput=False)
    wm_in = nc.declare_dram_parameter("wm", [4, 128, 512], f16, isOutput=False)
    c_in = nc.declare_dram_parameter("c", [BPC, 2, 128, NCHUNK], f16, isOutput=False)
    d_in = nc.declare_dram_parameter("d", [BPC, 2, 128, NCHUNK], f16, isOutput=False)
    out = nc.declare_dram_parameter("out", [BPC, F, L], f16, isOutput=True)

    CH0 = 516  # first-column chunk so the pipeline can start early

    with tile.TileContext(nc) as tc:
        with (
            tc.tile_pool(name="wtp", bufs=1) as wtp,
            tc.tile_pool(name="cdp", bufs=2) as cdp,
            tc.tile_pool(name="fp", bufs=2) as fp,
            tc.tile_pool(name="pcp", bufs=4, space="PSUM") as pcp,
            tc.tile_pool(name="psp", bufs=3, space="PSUM") as psp,
            tc.tile_pool(name="p512p", bufs=1, space="PSUM") as p512p,
            tc.tile_pool(name="sqp", bufs=3) as sqp,
            tc.tile_pool(name="sp", bufs=3) as sp,
            tc.tile_pool(name="stp", bufs=3) as stp,
            tc.tile_pool(name="r512p", bufs=2) as r512p,
        ):
            cd_sb = [None] * BPC

            def load_cd(b, lo, hi):
                if cd_sb[b] is None:
                    cd_sb[b] = (
                        [
                            cdp.tile([128, NCHUNK], f16, name=f"c{h}", tag=f"c{h}")
                            for h in range(2)
                        ],
                        [
                            cdp.tile([128, NCHUNK], f16, name=f"d{h}", tag=f"d{h}")
                            for h in range(2)
                        ],
                    )
                c_sb, d_sb = cd_sb[b]
                for h in range(2):
                    nc.sync.dma_start(c_sb[h][:, lo:hi], c_in[b, h, :, lo:hi])
                    nc.sync.dma_start(d_sb[h][:, lo:hi], d_in[b, h, :, lo:hi])

            # DMA ring order: batch-0 first chunk, cos weights, batch-0 rest,
            # sin weights — matches the order the PE consumes them.
            load_cd(0, 0, CH0)

            wp_sb, wm_sb = [], []
            for a in range(4):
                t = wtp.tile([128, 513], f16, name=f"wpa{a}")
                nc.sync.dma_start(t[:], wp_in[a])
                wp_sb.append(t)

            load_cd(0, CH0, NCHUNK)

            for a in range(4):
                t = wtp.tile([128, 512], f16, name=f"wma{a}")
                nc.sync.dma_start(t[:], wm_in[a])
                wm_sb.append(t)

            for b in range(BPC):
                c_sb, d_sb = cd_sb[b]
                fpl = [
                    fp.tile([128, L], f16, name=f"fp{a}", tag=f"fp{a}")
                    for a in range(4)
                ]
                fmi = [
                    fp.tile([128, L], f16, name=f"fm{a}", tag=f"fm{a}")
                    for a in range(4)
                ]

                def fold_cols(lo, hi):
                    # plus folds first (cos matmuls consume them first)
                    for sign in range(2):
                        for a in range(4):
                            g = a & 1
                            ao = a >> 1
                            cs = c_sb[g][:, lo + ao : hi + ao]
                            ds = d_sb[g][:, lo + 4 - ao : hi + 4 - ao]
                            nc.vector.tensor_tensor(
                                (fpl if sign == 0 else fmi)[a][:, lo:hi],
                                cs,
                                ds,
                                op=mybir.AluOpType.add
                                if sign == 0
                                else mybir.AluOpType.subtract,
                            )
                        # slot m=0 of both folds carries the self-paired center
                        # sample x[512]; its weight column is w2[:, 512], which
                        # is nonzero even for sin rows (f32 rounding of the
                        # reference angle leaves ~1e-4 there).
                        nc.vector.tensor_copy(
                            (fpl if sign == 0 else fmi)[0][0:1, lo:hi],
                            c_sb[0][0:1, lo + 2 : hi + 2],
                        )

                fold_cols(0, 512)
                fold_cols(512, L)

                for n in range(NT):
                    nsl = slice(n * 512, (n + 1) * 512)
                    if b + 1 < BPC and n == 0:
                        # batch-1 signal streams in while batch-0 computes
                        load_cd(b + 1, 0, CH0)
                        load_cd(b + 1, CH0, NCHUNK)
                    # interleave cos/sin per q so each pair's magnitude
                    # pipeline starts as early as possible
                    pc_t, ps_t = [], []
                    for q in range(QT):
                        pc = pcp.tile([128, 512], f32, name=f"pc{b}{n}{q}", tag="pc")
                        for a in range(4):
                            nc.tensor.matmul(
                                pc[:],
                                wp_sb[a][:, q * 128 : (q + 1) * 128],
                                fpl[a][:, nsl],
                                start=(a == 0),
                                stop=(a == 3),
                            )
                        pc_t.append(pc)
                        pss = psp.tile([128, 512], f32, name=f"psn{b}{n}{q}", tag="ps")
                        for a in range(4):
                            nc.tensor.matmul(
                                pss[:],
                                wm_sb[a][:, q * 128 : (q + 1) * 128],
                                fmi[a][:, nsl],
                                start=(a == 0),
                                stop=(a == 3),
                            )
                        ps_t.append(pss)
                    p512 = p512p.tile([1, 512], f32, name=f"p512{b}{n}", tag="p512")
                    for a in range(4):
                        nc.tensor.matmul(
                            p512[:],
                            wp_sb[a][:, 512:513],
                            fpl[a][:, nsl],
                            start=(a == 0),
                            stop=(a == 3),
                        )

                    r512 = r512p.tile([1, 512], f16, name=f"r512{b}{n}", tag="r512")
                    nc.scalar.activation(
                        r512[:], p512[:], mybir.ActivationFunctionType.Abs
                    )
                    nc.vector.tensor_scalar_max(r512[:], r512[:], 1e-6)
                    nc.gpsimd.dma_start(out[b, F - 1 : F, nsl], r512[:])

                    for q in range(QT):
                        sq_c = sqp.tile([128, 512], f16, name=f"sqc{b}{n}{q}", tag="sqc")
                        sq_s = sqp.tile([128, 512], f16, name=f"sqs{b}{n}{q}", tag="sqs")
                        if q == 3 and not (b == BPC - 1 and n == NT - 1):
                            # relieve the saturated ACT: square the last pair
                            # on DVE via fp16 PSUM copies (fp16 TT runs 2x)
                            cp_c = sqp.tile(
                                [128, 512], f16, name=f"cpc{b}{n}{q}", tag="cpc"
                            )
                            cp_s = sqp.tile(
                                [128, 512], f16, name=f"cps{b}{n}{q}", tag="cps"
                            )
                            nc.vector.tensor_copy(cp_c[:], pc_t[q][:])
                            nc.vector.tensor_copy(cp_s[:], ps_t[q][:])
                            nc.vector.tensor_tensor(
                                sq_c[:], cp_c[:], cp_c[:], op=mybir.AluOpType.mult
                            )
                            nc.vector.tensor_tensor(
                                sq_s[:], cp_s[:], cp_s[:], op=mybir.AluOpType.mult
                            )
                        else:
                            nc.scalar.square(sq_c[:], pc_t[q][:])
                            nc.scalar.square(sq_s[:], ps_t[q][:])
                        s = sp.tile([128, 512], f16, name=f"s{b}{n}{q}", tag="s")
                        # sin bin-0 row is zero, so row 0 automatically gives
                        # sqrt(max(re0^2, eps)) = mag of bin 0.  fp16 tiles:
                        # the STT runs in the DVE 2x perf mode.
                        nc.vector.tensor_tensor(
                            s[:], sq_c[:], sq_s[:], op=mybir.AluOpType.add
                        )
                        st = stp.tile([128, 512], f16, name=f"st{b}{n}{q}", tag="st")
                        nc.scalar.sqrt(st[:], s[:])
                        nc.sync.dma_start(out[b, q * 128 : (q + 1) * 128, nsl], st[:])
    nc.finalize()
    return nc


def _build_program_dif():
    """Radix-2 decimation-in-frequency variant, K=256, all folds on the host.

    Even bins 2a (a=0..255) are the 512-point DFT of u[n] = xw[n]+xw[n+512];
    odd bins 2a+1 come from v[n] = xw[n]-xw[n+512] against the odd-bin basis.
    Both halves fold again about the frame center (cos rows symmetric, sin
    antisymmetric), giving four K=256 fold vectors per frame:

      P  = win*A + wb*B   (even Re)     A  = x[m]+x[1024-m]   wb = 1-win
      Q  = win*A- + wb*B- (even Im)     A- = x[m]-x[1024-m]
      P' = win*A - wb*B   (odd Re)      B  = x[512-m]+x[512+m]
      Q' = win*A- - wb*B- (odd Im)      B- = x[512+m]-x[512-m]

    All of these are shifted-column sums of per-partition-scaled signal
    layouts, so the HOST builds them directly (same total bytes as the old
    C/D layouts) and the device does zero fold work.  Specials: P[0] = x[512]
    (weight 1 on every cos row), u256 = (x[256]+x[768])/2 enters cos-even
    rows with weight (-1)^a via a K=1 matmul, v256 likewise for sin-odd;
    bin 512 = |sum_m (-1)^m P[m] + u256| via an M=1 matmul into the unused
    a=0 row of the sin-even-q0 PSUM tile.

    Per (batch, 512-frame tile): 23 matmuls of N=512 (vs 36 in 'fold').
    Magnitude: ACT squares the cos PSUM pairs (fp16 out), DVE squares the
    sin pairs (TT psum*psum), DVE adds in fp16 (2x mode), ACT sqrts; POOL_SQ
    of the 4 cos squares ride the gpsimd engine instead.  Output rows are
    written as separate even/odd planes (f16) and interleaved on the host.
    """
    import concourse.bacc as bacc
    import concourse.mybir as mybir
    import concourse.tile as tile

    f32 = mybir.dt.float32
    f16 = mybir.dt.float16
    NT_ = NT
    NW = 16  # weight tiles packed in wall

    nc = bacc.Bacc("TRN2", target_bir_lowering=False, debug=False)
    p_in = nc.declare_dram_parameter("p", [BPC, 2, 128, L], f16, isOutput=False)
    q_in = nc.declare_dram_parameter("q", [BPC, 2, 128, L], f16, isOutput=False)
    pp_in = nc.declare_dram_parameter("pp", [BPC, 2, 128, L], f16, isOutput=False)
    qp_in = nc.declare_dram_parameter("qp", [BPC, 2, 128, L], f16, isOutput=False)
    uv_in = nc.declare_dram_parameter("uv", [BPC, 1, 2 * L], f16, isOutput=False)
    wall_in = nc.declare_dram_parameter("wall", [128, NW * 128 + 1], f16, isOutput=False)
    srow_in = nc.declare_dram_parameter("srow", [1, 129], f16, isOutput=False)
    # outi rows: 0..255 = even-bin mags (a), 256..511 = odd-bin mags,
    # 512 = bin-512 row.  Host interleaves.
    outi = nc.declare_dram_parameter("outi", [BPC, F, L], f16, isOutput=True)

    with tile.TileContext(nc) as tc:
        with (
            tc.tile_pool(name="wp", bufs=1) as wp,
            tc.tile_pool(name="sig", bufs=2) as sigp,
            tc.tile_pool(name="ps", bufs=7, space="PSUM") as psp,
            tc.tile_pool(name="p512p", bufs=1, space="PSUM") as p512p,
            tc.tile_pool(name="sqp", bufs=10) as sqp,
            tc.tile_pool(name="sp", bufs=3) as sp,
            tc.tile_pool(name="stp", bufs=3) as stp,
            tc.tile_pool(name="rp", bufs=2) as rp,
        ):
            sig_sb = []
            for b in range(BPC):
                sig_sb.append({
                    "P": [sigp.tile([128, L], f16, name=f"P{b}{h}", tag=f"P{h}") for h in range(2)],
                    "Q": [sigp.tile([128, L], f16, name=f"Q{b}{h}", tag=f"Q{h}") for h in range(2)],
                    "Pp": [sigp.tile([128, L], f16, name=f"Pp{b}{h}", tag=f"Pp{h}") for h in range(2)],
                    "Qp": [sigp.tile([128, L], f16, name=f"Qp{b}{h}", tag=f"Qp{h}") for h in range(2)],
                    "uv": sigp.tile([1, 2 * L], f16, name=f"uv{b}", tag="uv"),
                })

            wall = wp.tile([128, NW * 128 + 1], f16, name="wall")
            srow = wp.tile([1, 129], f16, name="srow")
            # Three parallel DMA queues (SP-HW, ACT-HW, gpsimd-SW), each FIFO.
            # Order by first use: E-pair tensors (P, Q) for both batches lead,
            # O-pair tensors (Pp, Qp) trail behind the first E computes.
            nc.sync.dma_start(wall[:, 0:512], wall_in[:, 0:512])
            nc.sync.dma_start(srow[:], srow_in[:])
            for h in range(2):
                nc.sync.dma_start(sig_sb[0]["P"][h][:], p_in[0, h])
                nc.scalar.dma_start(sig_sb[0]["Q"][h][:], q_in[0, h])
                nc.gpsimd.dma_start(sig_sb[0]["Pp"][h][:], pp_in[0, h])
            nc.gpsimd.dma_start(sig_sb[0]["uv"][:], uv_in[0])
            nc.sync.dma_start(wall[:, 512:], wall_in[:, 512:])
            for h in range(2):
                nc.gpsimd.dma_start(sig_sb[0]["Qp"][h][:], qp_in[0, h])
            for h in range(2):
                nc.sync.dma_start(sig_sb[1]["P"][h][:], p_in[1, h])
                nc.scalar.dma_start(sig_sb[1]["Q"][h][:], q_in[1, h])
            nc.gpsimd.dma_start(sig_sb[1]["uv"][:], uv_in[1])
            for h in range(2):
                nc.scalar.dma_start(sig_sb[1]["Pp"][h][:], pp_in[1, h])
                nc.gpsimd.dma_start(sig_sb[1]["Qp"][h][:], qp_in[1, h])

            def W(i):
                return wall[:, 128 * i : 128 * (i + 1)]

            # wall tile order: wce(q,h), wse(q,h), wco(q,h), wso(q,h); col 2048
            # is the (-1)^p bin-512 column; srow = [(-1)^j (128), one]
            WCE = lambda qq, h: W(0 + 2 * qq + h)
            WSE = lambda qq, h: W(4 + 2 * qq + h)
            WCO = lambda qq, h: W(8 + 2 * qq + h)
            WSO = lambda qq, h: W(12 + 2 * qq + h)
            W512 = wall[:, NW * 128 : NW * 128 + 1]
            SGN = srow[0:1, 0:128]
            ONE = srow[0:1, 128:129]

            rwide = {b: rp.tile([1, L], f16, name=f"rw{b}", tag="rw")
                     for b in range(BPC)}

            for b in range(BPC):
                for n in range(NT_):
                    sig = sig_sb[b]
                    Ph = sig["P"]
                    Qh = sig["Q"]
                    Pph = sig["Pp"]
                    Qph = sig["Qp"]
                    uv = sig["uv"]
                    nsl = slice(n * 512, (n + 1) * 512)
                    usl = slice(n * 512, (n + 1) * 512)

                    # ---- matmuls: 4 (pc, ps) psum pairs ----
                    def mm_cos_even(qq):
                        pc = psp.tile([128, 512], f32, name=f"pcE{qq}_{b}{n}", tag="ps")
                        for h in range(2):
                            nc.tensor.matmul(pc[:], WCE(qq, h), Ph[h][:, nsl],
                                             start=(h == 0), stop=False)
                        nc.tensor.matmul(pc[:], SGN, uv[0:1, usl], start=False, stop=True)
                        return pc

                    def mm_sin_even(qq):
                        # q=0 row 0 has all-zero weights (sin a=0), so the E0
                        # pair's row 0 gives sqrt(re0^2) = bin 0 for free
                        ps = psp.tile([128, 512], f32, name=f"psE{qq}_{b}{n}", tag="ps")
                        for h in range(2):
                            nc.tensor.matmul(ps[:], WSE(qq, h), Qh[h][:, nsl],
                                             start=(h == 0), stop=(h == 1))
                        return ps

                    def mm_bin512():
                        # bin 512 = |sum_m (-1)^m P[m] + u256|
                        p5 = p512p.tile([1, 512], f32, name=f"p512_{b}{n}", tag="p512")
                        for h in range(2):
                            nc.tensor.matmul(p5[:], W512, Ph[h][:, nsl],
                                             start=(h == 0), stop=False)
                        nc.tensor.matmul(p5[:], ONE, uv[0:1, usl], start=False, stop=True)
                        return p5

                    def mm_cos_odd(qq):
                        pc = psp.tile([128, 512], f32, name=f"pcO{qq}_{b}{n}", tag="ps")
                        for h in range(2):
                            nc.tensor.matmul(pc[:], WCO(qq, h), Pph[h][:, nsl],
                                             start=(h == 0), stop=(h == 1))
                        return pc

                    def mm_sin_odd(qq):
                        ps = psp.tile([128, 512], f32, name=f"psO{qq}_{b}{n}", tag="ps")
                        for h in range(2):
                            nc.tensor.matmul(ps[:], WSO(qq, h), Qph[h][:, nsl],
                                             start=(h == 0), stop=(h == 1))
                        return ps

                    # ---- magnitude ----
                    # PSUM reads: ACT any op, DVE max one PSUM operand, Pool
                    # none (verified against the BIR verifier).  ACT squares
                    # the cos tiles (plus alternate sin tiles) straight off
                    # PSUM; remaining sin tiles drain via DVE cast + fp16
                    # self-mult.  All four adds land on Pool (SBUF fp16).
                    # One [128,2048] sqrt per (b,n); its st tile maps to outi
                    # rows 0:512 as four 128-row blocks in a single DMA.
                    def sqtile(tag, p_, act_sq):
                        sq = sqp.tile([128, 512], f16, name=f"sq{tag}{b}{n}", tag="sq")
                        if act_sq:
                            nc.scalar.square(sq[:], p_[:])
                        else:
                            ca = sqp.tile([128, 512], f16, name=f"ca{tag}{b}{n}", tag="ca")
                            nc.vector.tensor_copy(ca[:], p_[:])
                            nc.vector.tensor_tensor(sq[:], ca[:], ca[:],
                                                    op=mybir.AluOpType.mult)
                        return sq

                    def magpair(tag, pc, ps, s, half, act_sin, pool_add):
                        sqc = sqtile("c" + tag, pc, True)
                        sqs = sqtile("s" + tag, ps, act_sin)
                        eng = nc.gpsimd if pool_add else nc.vector
                        eng.tensor_tensor(s[:, 512 * half : 512 * half + 512],
                                          sqc[:], sqs[:], op=mybir.AluOpType.add)

                    s4 = sp.tile([128, 2048], f16, name=f"s4{b}{n}", tag="s")
                    act_sin = ((b * NT_ + n) % 2) == 0

                    pcE0 = mm_cos_even(0)
                    psE0 = mm_sin_even(0)
                    magpair("E0", pcE0, psE0, s4, 0, act_sin=act_sin, pool_add=True)
                    p5 = mm_bin512()
                    nc.scalar.activation(rwide[b][0:1, nsl], p5[:],
                                         mybir.ActivationFunctionType.Abs)
                    pcE1 = mm_cos_even(1)
                    psE1 = mm_sin_even(1)
                    magpair("E1", pcE1, psE1, s4, 1, act_sin=False, pool_add=True)
                    pcO0 = mm_cos_odd(0)
                    psO0 = mm_sin_odd(0)
                    magpair("O0", pcO0, psO0, s4, 2, act_sin=False, pool_add=True)
                    pcO1 = mm_cos_odd(1)
                    psO1 = mm_sin_odd(1)
                    magpair("O1", pcO1, psO1, s4, 3, act_sin=False, pool_add=True)
                    st4 = stp.tile([128, 2048], f16, name=f"st4{b}{n}", tag="st")
                    nc.scalar.sqrt(st4[:], s4[:])
                    if MERGE_OUT:
                        dst = outi[b, 0:512, nsl].rearrange("(k p) t -> p k t", p=128)
                        nc.sync.dma_start(dst, st4[:].rearrange("p (k t) -> p k t", k=4))
                    else:
                        for k in range(4):
                            nc.sync.dma_start(outi[b, 128 * k : 128 * (k + 1), nsl],
                                              st4[:, 512 * k : 512 * (k + 1)])
            for b in range(BPC):
                nc.sync.dma_start(outi[b, F - 1 : F, :], rwide[b][:])
    nc.finalize()
    return nc


def _get_program():
    key = MODE
    if key not in _PROGRAM_CACHE:
        if MODE == "dif":
            _PROGRAM_CACHE[key] = _build_program_dif()
        elif MODE == "fold":
            _PROGRAM_CACHE[key] = _build_program_fold()
        else:
            _PROGRAM_CACHE[key] = _build_program()
    return _PROGRAM_CACHE[key]


def _make_weight_np():
    n = np.arange(N_FFT, dtype=np.float32)
    k = np.arange(N_FFT // 2 + 1, dtype=np.float32)[:, None]
    ang = (-2.0 * np.pi / N_FFT) * k * n[None, :]
    win = 0.5 * (1.0 - np.cos(2.0 * np.pi * n / N_FFT))
    return np.concatenate([np.cos(ang), np.sin(ang)], axis=0) * win  # [1026, 1024]


def _pack_weight(weight):
    if weight is None:
        w2 = _make_weight_np()
    else:
        w2 = np.asarray(weight, dtype=np.float32).reshape(2 * (N_FFT // 2 + 1), N_FFT)
    # rows: cos 0..511, cos 512, sin 1..511  (sin 0 and sin 512 are zero rows)
    w_eff = np.concatenate([w2[0:512], w2[512:513], w2[514:1025]], axis=0)
    # Wt[k, p, m] = w_eff[m, 128k + p]
    wt = np.ascontiguousarray(
        w_eff.T.reshape(KT, 128, N_FFT), dtype=np.float32
    ).astype(_np_w_dtype())
    return wt


def _frame_layout(xb):
    """[SAMPLES] f32 -> C[2, 128, NCHUNK] with C[h, p, c] = xp[256c + 128h + p]."""
    xp = np.empty(NCHUNK * HOP, dtype=np.float32)
    xp[:CACHE] = 0.0
    xp[CACHE : CACHE + SAMPLES] = xb
    xp[CACHE + SAMPLES :] = 0.0
    return np.ascontiguousarray(xp.reshape(NCHUNK, 2, 128).transpose(1, 2, 0))


def _frame_layout_rev(xb):
    """Partition-reversed copy: D[g, p, c] = xp[256c - 128g - p] (0 if oob)."""
    xp = np.empty(NCHUNK * HOP, dtype=np.float32)
    xp[:CACHE] = 0.0
    xp[CACHE : CACHE + SAMPLES] = xb
    xp[CACHE + SAMPLES :] = 0.0
    c = 256 * np.arange(NCHUNK, dtype=np.int64)[None, None, :]
    g = 128 * np.arange(2, dtype=np.int64)[:, None, None]
    p = np.arange(128, dtype=np.int64)[None, :, None]
    idx = c - g - p
    d = xp[np.clip(idx, 0, None)]
    d[idx < 0] = 0.0
    return np.ascontiguousarray(d)


def _pack_weight_fold(weight):
    if weight is None:
        w2 = _make_weight_np()
    else:
        w2 = np.asarray(weight, dtype=np.float32).reshape(2 * (N_FFT // 2 + 1), N_FFT)
    # fold column j contracts x[j] + x[1024-j] (j = 1..511); slot j=0 carries
    # the center sample x[512], whose weight column is w2[:, 512].
    colmap = np.concatenate([[512], np.arange(1, 512)])
    wplus = w2[0:513][:, colmap]  # cos bins 0..512  [513, 512]
    wminus = w2[513:1025][:, colmap]  # sin bins 0..511 (row 0 zero)  [512, 512]
    wp = np.ascontiguousarray(wplus.T.reshape(4, 128, 513)).astype(np.float16)
    wm = np.ascontiguousarray(wminus.T.reshape(4, 128, 512)).astype(np.float16)
    return wp, wm


def _host_prep_dif(xb):
    """[SAMPLES] f32 -> P, Q, Pp, Qp [2,128,L] f16 + uv row [1, 2L] f16."""
    from numpy.lib.stride_tricks import as_strided

    xp = np.zeros(NCHUNK * HOP, dtype=np.float32)
    xp[CACHE : CACHE + SAMPLES] = xb
    s = xp.strides[0]
    X0 = as_strided(xp, (256, L), (s, 256 * s))          # x[256t + m]
    X1024 = as_strided(xp[1024:], (256, L), (-s, 256 * s))  # x[256t + 1024 - m]
    X512p = as_strided(xp[512:], (256, L), (s, 256 * s))    # x[256t + 512 + m]
    X512m = as_strided(xp[512:], (256, L), (-s, 256 * s))   # x[256t + 512 - m]

    n256 = np.arange(256, dtype=np.float32)
    winm = (0.5 * (1.0 - np.cos(2.0 * np.pi * n256 / N_FFT)))[:, None]
    wbm = 1.0 - winm
    A = X0 + X1024
    Am = X0 - X1024
    B = X512m + X512p
    Bm = X512p - X512m
    wA = winm * A
    wB = wbm * B
    wAm = winm * Am
    wBm = wbm * Bm
    P = wA + wB
    Pp = wA - wB
    Q = wAm + wBm
    Qp = wAm - wBm
    u0 = xp[512 : 512 + HOP * L : HOP]
    P[0, :] = u0
    Pp[0, :] = -u0
    Q[0, :] = 0.0
    r256 = xp[256 : 256 + HOP * L : HOP]
    r768 = xp[768 : 768 + HOP * L : HOP]
    u256 = 0.5 * (r256 + r768)
    v256 = 0.5 * (r256 - r768)
    # Q' uses slots m = 1..256: the Nyquist slot m=256 has natural weight
    # sin(pi(2a+1)/2) = (-1)^a, exactly v256's pattern, so v256 rides free.
    Qp_s = np.vstack([Qp[1:256], v256[None, :]])
    uv = np.concatenate([u256, v256])
    f16 = np.float16
    rs = lambda z: np.ascontiguousarray(z.reshape(2, 128, L), dtype=f16)
    return rs(P), rs(Q), rs(Pp), rs(Qp_s), uv.astype(f16)[None, :]


def _pack_weights_dif():
    """wall [128, 16*128+1] f16 (lhsT tiles) + srow [1, 129] f16."""
    j = np.arange(128, dtype=np.float64)[None, :]
    p = np.arange(128, dtype=np.float64)[:, None]
    tiles = []
    for grp in range(4):  # wce, wse, wco, wso
        for q in (0, 1):
            for h in (0, 1):
                m = 128 * h + p
                if grp == 0:
                    t = np.cos(2 * np.pi * (128 * q + j) * m / 512)
                elif grp == 1:
                    t = np.sin(2 * np.pi * (128 * q + j) * m / 512)  # row a=0 zero
                elif grp == 2:
                    t = np.cos(2 * np.pi * (2 * (128 * q + j) + 1) * m / 1024)
                else:
                    t = np.sin(2 * np.pi * (2 * (128 * q + j) + 1) * (m + 1) / 1024)
                tiles.append(t)
    wall = np.empty((128, 16 * 128 + 1), dtype=np.float64)
    wall[:, : 16 * 128] = np.concatenate(tiles, axis=1)
    wall[:, 16 * 128] = (-1.0) ** np.arange(128)  # (-1)^m column for bin 512
    srow = np.empty((1, 129), dtype=np.float64)
    srow[0, :128] = (-1.0) ** np.arange(128)
    srow[0, 128] = 1.0
    return wall.astype(np.float16), srow.astype(np.float16)


def _in_maps_dif(x):
    wall, srow = _pack_weights_dif()
    maps = []
    for i in range(NCORES):
        P, Q, Pp, Qp, uv = [], [], [], [], []
        for b in range(BPC):
            pb, qb, ppb, qpb, uvb = _host_prep_dif(x[BPC * i + b])
            P.append(pb)
            Q.append(qb)
            Pp.append(ppb)
            Qp.append(qpb)
            uv.append(uvb)
        maps.append(
            {
                "p": np.stack(P),
                "q": np.stack(Q),
                "pp": np.stack(Pp),
                "qp": np.stack(Qp),
                "uv": np.stack(uv),
                "wall": wall,
                "srow": srow,
            }
        )
    return maps


def _assemble_dif(results):
    """Interleave even/odd f16 bin planes from each core into f32 output."""
    out = np.empty((BATCH, F, L), dtype=np.float32)
    for i in range(NCORES):
        outi = results[i]["outi"]
        for b in range(BPC):
            out[BPC * i + b, 0:512:2] = outi[b, 0:256]
            out[BPC * i + b, 1:512:2] = outi[b, 256:512]
            out[BPC * i + b, 512] = outi[b, 512]
    return out


def _in_maps(x, weight):
    if MODE == "dif":
        return _in_maps_dif(x)
    if MODE == "fold":
        wp, wm = _pack_weight_fold(weight)
        maps = []
        for i in range(NCORES):
            c = np.stack([_frame_layout(x[BPC * i + b]) for b in range(BPC)])
            d = np.stack([_frame_layout_rev(x[BPC * i + b]) for b in range(BPC)])
            maps.append(
                {"wp": wp, "wm": wm, "c": c.astype(np.float16), "d": d.astype(np.float16)}
            )
        return maps
    wt = _pack_weight(weight)
    maps = []
    for i in range(NCORES):
        c = np.stack([_frame_layout(x[BPC * i + b]) for b in range(BPC)])
        maps.append({"w": wt, "c": c})
    return maps


def kernel(x, weight=None, **_unused):
    from concourse.bass_utils import run_bass_kernel_spmd

    x = np.asarray(x, dtype=np.float32)
    assert x.shape == (BATCH, SAMPLES), x.shape

    nc = _get_program()
    res = run_bass_kernel_spmd(nc, _in_maps(x, weight), core_ids=list(range(NCORES)))

    if MODE == "dif":
        return _assemble_dif(res.results)
    out = np.empty((BATCH, F, L), dtype=np.float32)
    for i in range(NCORES):
        out[BPC * i : BPC * (i + 1)] = res.results[i]["out"]  # f16 -> f32
    return out

